# revision 1
# baseline (speedup 1.0000x reference)
"""GATNet (2x GATConv + MLP head + log_softmax) on 8 Trainium2 NeuronCores.

Strategy (dst-partitioned message passing):
  - Host assigns destination nodes to 8 devices x SPD slots (32 nodes/slot),
    balancing in-edge counts so every slot has exactly TPS 128-edge tiles.
    Every device runs an identical program; per-device data (edge shards,
    index arrays) differ.
  - Per layer, each device builds the full node table T = [h | a_s] (bf16 h,
    f32 a_s packed in 256B/node rows) for all nodes, then processes its own
    edge shard: per-edge rows are fetched with dma_gather over node PAIRS
    (512B elements, int16 pair indices), attention softmax is computed with
    the denominator deferred to the node level, source-parity is folded into
    even/odd exp-weight blocks, and messages are aggregated per 32-node slot
    with one-hot matmuls accumulating in PSUM.
  - a_d[dst] is expanded edge-wise by transposing the one-hot with the PE and
    multiplying against a block-diagonal table of the quad's a_d values.
  - Between the two GAT layers one AllGather shares the transposed layer-1
    node outputs.  The head MLP + log_softmax run on local nodes.

Numerics: the reference subtracts segment_max before exp purely for
stability; alpha here is O(1) so exp() is computed directly (softmax result
is mathematically identical).  h and the one-hot travel as bf16; a_s, a_e,
alpha, the PSUM accumulations and all node-level math stay fp32.
"""

import numpy as np

# model constants (fixed by the problem)
IN = 128
HID = 16
OUT = 40
H = 4
ED = 16
HC = 64  # HID * H
NEG = 0.2
EPS = 1e-16

C = 8          # NeuronCores
NSLOT = 32     # nodes per slot (= one-hot width, PSUM col-block)


# ----------------------------------------------------------------------------
# host-side plan: balance nodes into (device, slot) bins, lay out edge shards
# ----------------------------------------------------------------------------

def _build_plan(src, dst, n_nodes):
    """Returns a dict with the full sharding plan. src/dst include self-loops."""
    import heapq

    deg = np.bincount(dst, minlength=n_nodes).astype(np.int64)
    e_tot = src.shape[0]

    def try_pack(nbins, cap_e):
        # LPT: heaviest nodes first into least-loaded feasible bin
        order = np.argsort(-deg, kind="stable")
        loads = [(0, b) for b in range(nbins)]
        heapq.heapify(loads)
        bin_of_t = np.empty(n_nodes, np.int64)
        bin_cnt = np.zeros(nbins, np.int64)
        bin_load = np.zeros(nbins, np.int64)
        for nd in order:
            d = int(deg[nd])
            spill = []
            placed = False
            while loads:
                l, b = heapq.heappop(loads)
                if bin_cnt[b] < NSLOT and bin_load[b] + d <= cap_e:
                    bin_of_t[nd] = b
                    bin_cnt[b] += 1
                    bin_load[b] += d
                    heapq.heappush(loads, (bin_load[b], b))
                    placed = True
                    break
                elif bin_cnt[b] < NSLOT:
                    spill.append((l, b))
                # full bins are dropped
            for it in spill:
                heapq.heappush(loads, it)
            if not placed:
                return None
        return bin_of_t

    # search (slots-per-device, even tiles-per-slot) minimizing total tiles
    spd_min = 4 * int(np.ceil(n_nodes / (C * NSLOT * 4)))  # node-capacity floor
    best = None  # (tq, spd, tps, bin_of)
    for spd_try in range(spd_min, spd_min + 65, 4):
        nbins = C * spd_try
        tps_lo = int(np.ceil(e_tot / nbins / 128.0))
        tps_lo += tps_lo % 2  # ch = 4*tps must be a multiple of 8
        for tps_try in (tps_lo, tps_lo + 2):
            if best is not None and spd_try * tps_try >= best[0]:
                continue
            got = try_pack(nbins, tps_try * 128)
            if got is not None:
                best = (spd_try * tps_try, spd_try, tps_try, got)
                break
        if best is not None and (spd_try + 4) * 2 >= best[0]:
            break
    assert best is not None, "balancer failed"
    _, spd, tps, bin_of = best

    nbins = C * spd
    npd = spd * NSLOT
    ng = C * npd
    assert ng // 2 <= 32767, "pair index must fit int16"

    # position of each node within its bin
    pos_of = np.zeros(n_nodes, np.int64)
    fill = np.zeros(nbins, np.int64)
    for nd in range(n_nodes):
        b = bin_of[nd]
        pos_of[nd] = fill[b]
        fill[b] += 1
    dev_of_bin = np.arange(nbins) // spd
    ls_of_bin = np.arange(nbins) % spd
    node2g = (dev_of_bin[bin_of] * npd + ls_of_bin[bin_of] * NSLOT + pos_of).astype(np.int64)

    # edges sorted by destination bin; rank within bin
    ebin = bin_of[dst]
    order = np.argsort(ebin, kind="stable")
    counts = np.bincount(ebin, minlength=nbins)
    cap = tps * 128
    assert counts.max() <= cap
    starts = np.zeros(nbins + 1, np.int64)
    np.cumsum(counts, out=starts[1:])
    rank = np.arange(e_tot, dtype=np.int64) - starts[ebin[order]]
    canvas = np.full((nbins, cap), -1, np.int64)       # edge id or -1 pad
    canvas[ebin[order], rank] = order

    return dict(
        spd=spd, tps=tps, npd=npd, ng=ng, nbins=nbins,
        bin_of=bin_of, pos_of=pos_of, node2g=node2g, canvas=canvas,
    )


def _host_arrays(plan, x, src, dst, edge_attr, mean_attr, n_nodes):
    """Per-core input arrays."""
    spd, tps, npd, ng = plan["spd"], plan["tps"], plan["npd"], plan["ng"]
    node2g, pos_of, canvas = plan["node2g"], plan["pos_of"], plan["canvas"]
    tq = spd * tps                       # 128-edge tiles per device
    e0 = edge_attr.shape[0]

    # permuted node features, transposed: xT [IN, ng]
    xg = np.zeros((ng, IN), np.float32)
    xg[node2g] = np.asarray(x, np.float32)
    xT = np.ascontiguousarray(xg.T)

    per_core = []
    for d in range(C):
        cv = canvas[d * spd:(d + 1) * spd].reshape(tq, 128)  # [tile, lane]
        valid = cv >= 0
        eid = np.where(valid, cv, 0)
        srcg = np.where(valid, node2g[src[eid]], 0)
        # pair index (int16, wrapped in 16 partitions: idx i at [i%16, i//16])
        srcp = (srcg >> 1).astype(np.int16).reshape(tq * 128)   # i = t*128+p
        srcp_w = np.ascontiguousarray(np.tile(srcp.reshape(-1, 16).T, (8, 1)))  # [128, tq*8]
        par = np.ascontiguousarray((srcg & 1).astype(np.float32).T)      # [128, tq]
        drel = np.ascontiguousarray(
            np.where(valid, pos_of[dst[eid]].astype(np.float32), -1.0).astype(np.float32).T)
        ea = np.zeros((tq, 128, ED), np.float32)
        sel = valid & (eid < e0)
        ea[sel] = edge_attr[eid[sel]]
        loop_sel = valid & (eid >= e0)
        ea[loop_sel] = mean_attr
        # eaTg[(tt)*16 + r, g*128 + p]: groups of 8 tiles
        eaTg = np.ascontiguousarray(
            ea.reshape(tq // 8, 8, 128, ED).transpose(1, 3, 0, 2).reshape(128, (tq // 8) * 128))
        per_core.append(dict(
            srcp=srcp_w, par=par, dstrel=drel, eaTg=eaTg,
            xTloc=np.ascontiguousarray(xT[:, d * npd:(d + 1) * npd]),
        ))
    return per_core, xT, tq


def _fold_weights(W1, att_s1, att_d1, We1, att_e1, b1,
                  W2, att_s2, att_d2, We2, att_e2, b2,
                  lw1, lb1, lw2, lb2):
    def head_fold(att):  # [H, HID] -> [HC, H] block diag columns
        A = np.zeros((HC, H), np.float32)
        for h in range(H):
            A[h * HID:(h + 1) * HID, h] = att[h]
        return A

    W1aug = np.concatenate([W1, W1 @ head_fold(att_s1), W1 @ head_fold(att_d1)], 1).astype(np.float32)
    W2aug = np.concatenate([W2, W2 @ head_fold(att_s2), W2 @ head_fold(att_d2)], 1).astype(np.float32)
    Ve = np.zeros((ED, 8), np.float32)
    for h in range(H):
        Ve[:, h] = We1[:, h * HID:(h + 1) * HID] @ att_e1[h]
        Ve[:, 4 + h] = We2[:, h * HID:(h + 1) * HID] @ att_e2[h]
    VeBD = np.zeros((128, 64), np.float32)
    for j in range(8):
        VeBD[ED * j:ED * (j + 1), 8 * j:8 * (j + 1)] = Ve
    LW = (lw1 @ lw2).astype(np.float32)
    lb2p = (lb1 @ lw2 + lb2).astype(np.float32)
    return W1aug, W2aug, VeBD, LW, lb2p, b1.astype(np.float32), b2.astype(np.float32)


# ----------------------------------------------------------------------------
# the bass program (identical for all cores)
# ----------------------------------------------------------------------------

def _build_nc(ng, npd, spd, tps, tq, sim_safe=False):
    import concourse.bass as bass
    import concourse.mybir as mybir
    import concourse.tile as tile
    from concourse import bacc
    from contextlib import ExitStack

    F32 = mybir.dt.float32
    BF16 = mybir.dt.bfloat16
    I32 = mybir.dt.int32
    I16 = mybir.dt.int16
    ALU = mybir.AluOpType
    ACT = mybir.ActivationFunctionType

    ch = 4 * tps          # tiles per chunk (one quad = 4 slots)
    qpd = spd // 4        # chunks per device per layer
    nt = ng // 128        # node tiles (table build)
    jpd = npd // 128      # local 128-node groups
    ngr = ch // 8         # eaTg groups per chunk

    nc = bacc.Bacc(None, target_bir_lowering=False)

    # kernel IO
    t_xT = nc.dram_tensor("xT", [128, ng], F32, kind="ExternalInput")
    t_xTl = nc.dram_tensor("xTloc", [128, npd], F32, kind="ExternalInput")
    t_eaTg = nc.dram_tensor("eaTg", [128, (tq // 8) * 128], F32, kind="ExternalInput")
    t_srcp = nc.dram_tensor("srcp", [128, tq * 8], I16, kind="ExternalInput")
    t_par = nc.dram_tensor("par", [128, tq], F32, kind="ExternalInput")
    t_drel = nc.dram_tensor("dstrel", [128, tq], F32, kind="ExternalInput")
    t_W1 = nc.dram_tensor("W1aug", [128, 72], F32, kind="ExternalInput")
    t_W2 = nc.dram_tensor("W2aug", [64, 72], F32, kind="ExternalInput")
    t_VeBD = nc.dram_tensor("VeBD", [128, 64], F32, kind="ExternalInput")
    t_LW = nc.dram_tensor("LW", [64, OUT], F32, kind="ExternalInput")
    t_cst = nc.dram_tensor("cst", [1, 256], F32, kind="ExternalInput")
    # cst row: [b1(64) | b2(64) | lb2p(40) | iota32(32) | pad]
    t_I = nc.dram_tensor("ident", [128, 128], F32, kind="ExternalInput")
    t_out = nc.dram_tensor("out", [npd, OUT], F32, kind="ExternalOutput")

    # internal DRAM.  Node tables: 256B per node pair (bf16-typed bytes).
    d_T1 = nc.dram_tensor("T1", [ng // 2, 256], BF16)
    d_T2 = nc.dram_tensor("T2", [ng // 2, 256], BF16)
    d_ad1 = nc.dram_tensor("ad1", [npd, 4], F32)
    d_ad2 = nc.dram_tensor("ad2", [npd, 4], F32)
    d_ae2 = nc.dram_tensor("ae2", [128, tq * 4], F32)
    d_h1T = nc.dram_tensor("h1Tloc", [64, npd], F32)
    d_h1all = nc.dram_tensor("h1Tall", [C * 64, npd], F32, addr_space="Shared")

    def rows(tbl):  # [ng, 128] bf16 row view of the pair table
        return tbl.ap().rearrange("m (two d) -> (m two) d", two=2)

    with tile.TileContext(nc) as tc, ExitStack() as top:
        cp = top.enter_context(tc.tile_pool(name="consts", bufs=1))

        W1sb = cp.tile([128, 72], F32)
        W2sb = cp.tile([64, 72], F32)
        VeBD = cp.tile([128, 64], F32)
        LWsb = cp.tile([64, OUT], F32)
        Isb = cp.tile([128, 128], F32)
        b1bc = cp.tile([128, 64], F32)
        b2bc = cp.tile([128, 64], F32)
        lbbc = cp.tile([128, OUT], F32)
        iota = cp.tile([128, NSLOT], F32)
        Ib16 = cp.tile([128, 128], BF16)
        nc.sync.dma_start(W1sb[:], t_W1[:, :])
        nc.sync.dma_start(W2sb[:], t_W2[:, :])
        nc.sync.dma_start(VeBD[:], t_VeBD[:, :])
        nc.sync.dma_start(LWsb[:], t_LW[:, :])
        nc.sync.dma_start(Isb[:], t_I[:, :])
        nc.sync.dma_start(b1bc[:], t_cst[:, 0:64].partition_broadcast(128))
        nc.sync.dma_start(b2bc[:], t_cst[:, 64:128].partition_broadcast(128))
        nc.sync.dma_start(lbbc[:], t_cst[:, 128:128 + OUT].partition_broadcast(128))
        nc.sync.dma_start(iota[:], t_cst[:, 168:168 + NSLOT].partition_broadcast(128))
        nc.vector.tensor_copy(out=Ib16[:], in_=Isb[:])

        def table_write_batch(ph_sb, tbl, i0, bs, ps_list):
            """Write node rows [128*i0, 128*(i0+bs)) from bs psums [128, 72]."""
            hsb = ph_sb.tile([128, 4, 128], BF16, tag="hsb")
            for c in range(bs):
                nc.scalar.activation(hsb[:, c, 0:68], ps_list[c][:, 0:68], ACT.Copy)
            if sim_safe:
                nc.gpsimd.memset(hsb[:, 0:bs, 68:128], 0)
            rv = rows(tbl)
            nc.sync.dma_start(
                rv[128 * i0:128 * (i0 + bs), :].rearrange("(c r) d -> r c d", c=bs),
                hsb[:, 0:bs, :])

        # ---------------- phase A1: T1 = [x@W1 | a_s1]; local a_d1 ----------
        with ExitStack() as ph:
            ap = ph.enter_context(tc.tile_pool(name="pa_sb", bufs=3))
            app = ph.enter_context(tc.tile_pool(name="pa_ps", bufs=2, space="PSUM"))
            for i0 in range(0, nt, 4):
                bs = min(4, nt - i0)
                xt = ap.tile([128, 4 * 128], F32, tag="xt")
                nc.sync.dma_start(xt[:, 0:128 * bs], t_xT[:, 128 * i0:128 * (i0 + bs)])
                ps_list = []
                for c in range(bs):
                    ps = app.tile([128, 72], F32, tag=f"ps{c}")
                    nc.tensor.matmul(ps[:], xt[:, 128 * c:128 * (c + 1)], W1sb[:],
                                     start=True, stop=True)
                    ps_list.append(ps)
                table_write_batch(ap, d_T1, i0, bs, ps_list)
            for i0 in range(0, jpd, 4):
                bs = min(4, jpd - i0)
                xt = ap.tile([128, 4 * 128], F32, tag="xt")
                nc.sync.dma_start(xt[:, 0:128 * bs], t_xTl[:, 128 * i0:128 * (i0 + bs)])
                adb = ap.tile([128, 4, 4], F32, tag="adb")
                for c in range(bs):
                    ps = app.tile([128, 72], F32, tag=f"ps{c}")
                    nc.tensor.matmul(ps[:, 0:4], xt[:, 128 * c:128 * (c + 1)],
                                     W1sb[:, 68:72], start=True, stop=True)
                    nc.vector.tensor_copy(out=adb[:, c, :], in_=ps[:, 0:4])
                nc.sync.dma_start(
                    d_ad1[128 * i0:128 * (i0 + bs), :].rearrange("(c r) d -> r c d", c=bs),
                    adb[:, 0:bs, :])

        # ---------------- edge phase (shared for both layers) ----------------
        def edge_layer(layer, tbl, adt):
            with ExitStack() as ph:
                ip = ph.enter_context(tc.tile_pool(name=f"l{layer}_idx", bufs=3))
                gp = ph.enter_context(tc.tile_pool(name=f"l{layer}_g", bufs=3))
                sp = ph.enter_context(tc.tile_pool(name=f"l{layer}_s", bufs=2))
                mp = ph.enter_context(tc.tile_pool(name=f"l{layer}_m", bufs=2))
                ep = ph.enter_context(tc.tile_pool(name=f"l{layer}_e", bufs=2))
                pp = ph.enter_context(tc.tile_pool(name=f"l{layer}_ps", bufs=2, space="PSUM"))
                p1 = ph.enter_context(tc.tile_pool(name=f"l{layer}_p1", bufs=1, space="PSUM"))

                for q in range(qpd):
                    c0 = ch * q
                    # --- gather node pairs for the chunk's 4096*tps/8 edges
                    idx = ip.tile([128, ch * 8], I16, tag="idx")
                    nc.sync.dma_start(idx[:], t_srcp[:, c0 * 8:(c0 + ch) * 8])
                    g2 = gp.tile([128, ch, 256], BF16, tag="g2")
                    nc.gpsimd.dma_gather(
                        out_ap=g2[:], in_ap=tbl[:, :], idxs_ap=idx[:],
                        num_idxs=ch * 128, num_idxs_reg=ch * 128, elem_size=256,
                        single_packet=False)
                    par = ip.tile([128, ch], F32, tag="par")
                    nc.sync.dma_start(par[:], t_par[:, c0:c0 + ch])
                    drel = ip.tile([128, ch], F32, tag="drel")
                    nc.sync.dma_start(drel[:], t_drel[:, c0:c0 + ch])

                    # --- one-hot S, batch-major: [128, tps(b), 4(j), NSLOT]
                    S = sp.tile([128, tps, 4, NSLOT], BF16, tag="S")
                    nc.vector.tensor_tensor(
                        out=S[:],
                        in0=drel[:].rearrange("p (j b) -> p b j", b=tps)
                            .unsqueeze(3).to_broadcast([128, tps, 4, NSLOT]),
                        in1=iota[:].unsqueeze(1).unsqueeze(1)
                            .to_broadcast([128, tps, 4, NSLOT]),
                        op=ALU.is_equal)

                    # --- a_d expansion: S^T via PE, then block-diag matmul
                    adq = ip.tile([128, 4], F32, tag="adq")
                    nc.sync.dma_start(adq[:], adt[128 * q:128 * (q + 1), :])
                    bd = ip.tile([128, 16], F32, tag="bd")
                    nc.gpsimd.memset(bd[:], 0.0)
                    for j in range(4):
                        nc.vector.tensor_copy(out=bd[NSLOT * j:NSLOT * (j + 1), 4 * j:4 * (j + 1)],
                                              in_=adq[NSLOT * j:NSLOT * (j + 1), :])
                    alad = p1.tile([128, tps * 16], F32, tag="alad")
                    for b0 in range(0, tps, 2):
                        stp = p1.tile([128, 256], BF16, tag="stp")
                        for b in (b0, b0 + 1):
                            nc.tensor.transpose(
                                stp[:, 128 * (b - b0):128 * (b - b0 + 1)],
                                S[:, b, :, :].rearrange("p a w -> p (a w)"), Ib16[:])
                        sts = sp.tile([128, 256], F32, tag="sts")
                        nc.scalar.activation(sts[:], stp[:], ACT.Copy)
                        for b in (b0, b0 + 1):
                            nc.tensor.matmul(alad[:, 16 * b:16 * (b + 1)],
                                             sts[:, 128 * (b - b0):128 * (b - b0 + 1)],
                                             bd[:], start=True, stop=True)

                    # --- a_e
                    if layer == 1:
                        eac = ip.tile([128, 128 * ngr], F32, tag="eac")
                        nc.sync.dma_start(eac[:], t_eaTg[:, 128 * ngr * q:128 * ngr * (q + 1)])
                        cmp = ip.tile([128, ch, 4], F32, tag="cmp")
                        ae_tiles = []
                        for gi in range(ngr):
                            aeT = p1.tile([64, 128], F32, tag="aeT")
                            nc.tensor.matmul(aeT[:], VeBD[:], eac[:, 128 * gi:128 * (gi + 1)],
                                             start=True, stop=True)
                            aeTs = ep.tile([64, 128], F32, tag="aeTs")
                            nc.scalar.activation(aeTs[:], aeT[:], ACT.Copy)
                            aeps = p1.tile([128, 64], F32, tag="aeps")
                            nc.tensor.matmul(aeps[:], aeTs[:], Isb[0:64, 0:64], start=True, stop=True)
                            aesb = ep.tile([128, 64], F32, tag="aesb")
                            nc.scalar.activation(aesb[:], aeps[:], ACT.Copy)
                            ae_tiles.append(aesb)
                            nc.vector.tensor_copy(
                                out=cmp[:, 8 * gi:8 * (gi + 1), :],
                                in_=aesb[:].rearrange("p (j v) -> p j v", j=8)[:, :, 4:8])
                        nc.sync.dma_start(
                            d_ae2[:, 4 * ch * q:4 * ch * (q + 1)],
                            cmp[:].rearrange("p t v -> p (t v)"))

                    # --- alpha = a_s[src](parity-sel) + a_d[dst] + a_e
                    al = mp.tile([128, ch, 4], F32, tag="al")
                    nc.vector.tensor_tensor(out=al[:], in0=g2[:, :, 192:196],
                                            in1=g2[:, :, 64:68], op=ALU.subtract)
                    nc.vector.tensor_tensor(
                        out=al[:], in0=al[:],
                        in1=par[:].unsqueeze(2).to_broadcast([128, ch, 4]), op=ALU.mult)
                    nc.vector.tensor_tensor(out=al[:], in0=al[:], in1=g2[:, :, 64:68], op=ALU.add)
                    if layer == 1:
                        for gi in range(ngr):
                            nc.vector.tensor_tensor(
                                out=al[:, 8 * gi:8 * (gi + 1), :],
                                in0=al[:, 8 * gi:8 * (gi + 1), :],
                                in1=ae_tiles[gi][:].rearrange("p (j v) -> p j v", j=8)[:, :, 0:4],
                                op=ALU.add)
                    else:
                        ae2 = ip.tile([128, ch, 4], F32, tag="ae2")
                        nc.sync.dma_start(
                            ae2[:].rearrange("p t v -> p (t v)"),
                            d_ae2[:, 4 * ch * q:4 * ch * (q + 1)])
                        nc.vector.tensor_tensor(out=al[:], in0=al[:], in1=ae2[:], op=ALU.add)
                    # += a_d (tile (j, b) lives at alad[:, 16b + 4j : +4])
                    nc.vector.tensor_tensor(
                        out=al[:],
                        in0=al[:],
                        in1=alad[:].rearrange("p (b j v) -> p j b v", j=4, v=4),
                        op=ALU.add)
                    # leaky relu + exp
                    lk = mp.tile([128, ch, 4], F32, tag="lk")
                    nc.vector.tensor_scalar_mul(lk[:], al[:], NEG)
                    nc.vector.tensor_tensor(out=lk[:], in0=al[:], in1=lk[:], op=ALU.max)
                    ex = mp.tile([128, ch, 4], F32, tag="ex")
                    nc.scalar.activation(ex[:], lk[:], ACT.Exp)
                    # parity-split exp weights (bf16)
                    exo = mp.tile([128, ch, 4], F32, tag="exo")
                    nc.vector.tensor_tensor(
                        out=exo[:], in0=ex[:],
                        in1=par[:].unsqueeze(2).to_broadcast([128, ch, 4]), op=ALU.mult)
                    exe = mp.tile([128, ch, 4], F32, tag="exe")
                    nc.vector.tensor_tensor(out=exe[:], in0=ex[:], in1=exo[:], op=ALU.subtract)
                    exeb = mp.tile([128, ch, 4], BF16, tag="exeb")
                    nc.vector.tensor_copy(out=exeb[:], in_=exe[:])
                    exob = mp.tile([128, ch, 4], BF16, tag="exob")
                    nc.vector.tensor_copy(out=exob[:], in_=exo[:])
                    exb = mp.tile([128, ch, 4], BF16, tag="exb")
                    nc.vector.tensor_copy(out=exb[:], in_=ex[:])

                    # --- rhs blocks: msgE = h_even*exe, msgO = h_odd*exo
                    msgE = mp.tile([128, ch, 68], BF16, tag="msgE")
                    msgO = mp.tile([128, ch, 64], BF16, tag="msgO")
                    nc.vector.tensor_tensor(
                        out=msgE[:, :, 0:64].rearrange("p t (h c) -> p t h c", h=H),
                        in0=g2[:, :, 0:64].rearrange("p t (h c) -> p t h c", h=H),
                        in1=exeb[:].unsqueeze(3).to_broadcast([128, ch, H, HID]),
                        op=ALU.mult)
                    nc.vector.tensor_tensor(
                        out=msgO[:].rearrange("p t (h c) -> p t h c", h=H),
                        in0=g2[:, :, 128:192].rearrange("p t (h c) -> p t h c", h=H),
                        in1=exob[:].unsqueeze(3).to_broadcast([128, ch, H, HID]),
                        op=ALU.mult)
                    nc.vector.tensor_copy(out=msgE[:, :, 64:68], in_=exb[:])

                    # --- aggregate per slot into U4 (even+ex) and U4o (odd)
                    U4 = pp.tile([NSLOT, 4 * 68], F32, tag="U4")
                    U4o = pp.tile([NSLOT, 4 * 64], F32, tag="U4o")
                    for j in range(4):
                        for tt in range(tps):
                            t = tps * j + tt
                            nc.tensor.matmul(U4[:, 68 * j:68 * (j + 1)], S[:, tt, j, :], msgE[:, t, :],
                                             start=(tt == 0), stop=(tt == tps - 1))
                            nc.tensor.matmul(U4o[:, 64 * j:64 * (j + 1)], S[:, tt, j, :], msgO[:, t, :],
                                             start=(tt == 0), stop=(tt == tps - 1))

                    # --- epilogue: out = (UE+UO)/(den+eps) +bias, relu, ...
                    Usb = ep.tile([NSLOT, 4, 68], F32, tag="Usb")
                    nc.vector.tensor_copy(out=Usb[:], in_=U4[:].rearrange("p (j d) -> p j d", j=4))
                    nc.vector.tensor_tensor(
                        out=Usb[:, :, 0:64],
                        in0=Usb[:, :, 0:64],
                        in1=U4o[:].rearrange("p (j d) -> p j d", j=4),
                        op=ALU.add)
                    rec = ep.tile([NSLOT, 4, 4], F32, tag="rec")
                    nc.vector.tensor_scalar_add(rec[:], Usb[:, :, 64:68], EPS)
                    nc.vector.reciprocal(rec[:], rec[:])
                    outq = ep.tile([128, 64], F32, tag="outq")
                    for j in range(4):
                        nc.vector.tensor_tensor(
                            out=outq[NSLOT * j:NSLOT * (j + 1), :].rearrange("p (h c) -> p h c", h=H),
                            in0=Usb[:, j, 0:64].rearrange("p (h c) -> p h c", h=H),
                            in1=rec[:, j, :].unsqueeze(2).to_broadcast([NSLOT, H, HID]),
                            op=ALU.mult)
                    bias = b1bc if layer == 1 else b2bc
                    nc.vector.tensor_tensor(out=outq[:], in0=outq[:], in1=bias[:], op=ALU.add)
                    nc.scalar.activation(outq[:], outq[:], ACT.Relu)

                    # transpose out_quad (both layers need it)
                    tp = p1.tile([128, 128], F32, tag="aeps")
                    nc.tensor.transpose(tp[0:64, :], outq[:], Isb[:])
                    tpsb = ep.tile([64, 128], F32, tag="tpsb")
                    nc.scalar.activation(tpsb[:], tp[0:64, :], ACT.Copy)

                    if layer == 1:
                        nc.sync.dma_start(d_h1T[:, 128 * q:128 * (q + 1)], tpsb[:])
                        # local a_d2 for layer 2: relu(out1) @ wd2
                        adp = p1.tile([128, 4], F32, tag="aeT")
                        nc.tensor.matmul(adp[:], tpsb[:], W2sb[:, 68:72], start=True, stop=True)
                        adsb = ep.tile([128, 4], F32, tag="adsb")
                        nc.vector.tensor_copy(out=adsb[:], in_=adp[:])
                        nc.sync.dma_start(d_ad2[128 * q:128 * (q + 1), :], adsb[:])
                    else:
                        # head: logits = out2 @ (lw1@lw2) + lb2p ; log_softmax
                        lg = p1.tile([128, OUT], F32, tag="aeT")
                        nc.tensor.matmul(lg[:], tpsb[:], LWsb[:], start=True, stop=True)
                        z = ep.tile([128, OUT], F32, tag="z")
                        nc.vector.tensor_tensor(out=z[:], in0=lg[:], in1=lbbc[:], op=ALU.add)
                        mx = ep.tile([128, 1], F32, tag="mx")
                        nc.vector.reduce_max(mx[:], z[:], axis=mybir.AxisListType.X)
                        nc.vector.tensor_scalar(out=z[:], in0=z[:], scalar1=mx[:],
                                                scalar2=None, op0=ALU.subtract)
                        ez = ep.tile([128, OUT], F32, tag="ez")
                        nc.scalar.activation(ez[:], z[:], ACT.Exp)
                        sm = ep.tile([128, 1], F32, tag="sm")
                        nc.vector.reduce_sum(sm[:], ez[:], axis=mybir.AxisListType.X)
                        nc.scalar.activation(sm[:], sm[:], ACT.Ln)
                        nc.vector.tensor_scalar(out=z[:], in0=z[:], scalar1=sm[:],
                                                scalar2=None, op0=ALU.subtract)
                        nc.sync.dma_start(t_out[128 * q:128 * (q + 1), :], z[:])

        edge_layer(1, d_T1, d_ad1)

        # ---------------- AllGather of transposed layer-1 outputs ------------
        import concourse.mybir as _mb
        nc.gpsimd.collective_compute(
            "AllGather", _mb.AluOpType.bypass,
            replica_groups=[list(range(C))],
            ins=[d_h1T.ap().opt()],
            outs=[d_h1all.ap().opt()],
        )

        # ---------------- phase A2: T2 = [h1@W2 | a_s2] ----------------------
        with ExitStack() as ph:
            ap = ph.enter_context(tc.tile_pool(name="pb_sb", bufs=3))
            app = ph.enter_context(tc.tile_pool(name="pb_ps", bufs=2, space="PSUM"))
            for r in range(C):
                for jj0 in range(0, jpd, 4):
                    bs = min(4, jpd - jj0)
                    ht = ap.tile([64, 4 * 128], F32, tag="ht")
                    nc.sync.dma_start(ht[:, 0:128 * bs],
                                      d_h1all[64 * r:64 * (r + 1), 128 * jj0:128 * (jj0 + bs)])
                    ps_list = []
                    for c in range(bs):
                        ps = app.tile([128, 72], F32, tag=f"ps{c}")
                        nc.tensor.matmul(ps[:], ht[:, 128 * c:128 * (c + 1)], W2sb[:],
                                         start=True, stop=True)
                        ps_list.append(ps)
                    table_write_batch(ap, d_T2, r * jpd + jj0, bs, ps_list)

        edge_layer(2, d_T2, d_ad2)

    return nc


# ----------------------------------------------------------------------------
# public entry
# ----------------------------------------------------------------------------

def _prepare(inputs):
    x = np.asarray(inputs["x"], np.float32)
    ei = np.asarray(inputs["edge_index"], np.int64)
    ea = np.asarray(inputs["edge_attr"], np.float32)
    n = x.shape[0]
    loop = np.arange(n, dtype=np.int64)
    src = np.concatenate([ei[0], loop])
    dst = np.concatenate([ei[1], loop])
    mean_attr = ea.mean(axis=0)

    plan = _build_plan(src, dst, n)
    per_core, xT, tq = _host_arrays(plan, x, src, dst, ea, mean_attr, n)

    W1aug, W2aug, VeBD, LW, lb2p, b1, b2 = _fold_weights(
        np.asarray(inputs["W1"], np.float32), np.asarray(inputs["att_src1"], np.float32),
        np.asarray(inputs["att_dst1"], np.float32), np.asarray(inputs["We1"], np.float32),
        np.asarray(inputs["att_e1"], np.float32), np.asarray(inputs["b1"], np.float32),
        np.asarray(inputs["W2"], np.float32), np.asarray(inputs["att_src2"], np.float32),
        np.asarray(inputs["att_dst2"], np.float32), np.asarray(inputs["We2"], np.float32),
        np.asarray(inputs["att_e2"], np.float32), np.asarray(inputs["b2"], np.float32),
        np.asarray(inputs["lw1"], np.float32), np.asarray(inputs["lb1"], np.float32),
        np.asarray(inputs["lw2"], np.float32), np.asarray(inputs["lb2"], np.float32))

    cst = np.zeros((1, 256), np.float32)
    cst[0, 0:64] = b1
    cst[0, 64:128] = b2
    cst[0, 128:128 + OUT] = lb2p
    cst[0, 168:168 + NSLOT] = np.arange(NSLOT, dtype=np.float32)
    ident = np.eye(128, dtype=np.float32)

    in_maps = []
    for d in range(C):
        pc = per_core[d]
        in_maps.append({
            "xT": xT, "xTloc": pc["xTloc"], "eaTg": pc["eaTg"], "srcp": pc["srcp"],
            "par": pc["par"], "dstrel": pc["dstrel"], "W1aug": W1aug, "W2aug": W2aug,
            "VeBD": VeBD, "LW": LW, "cst": cst, "ident": ident,
        })
    return plan, in_maps, tq


def _assemble(plan, outs, n):
    node2g = plan["node2g"]
    full = np.concatenate([np.asarray(o, np.float32) for o in outs], axis=0)  # [ng, OUT]
    return full[node2g[:n]]


def _run(inputs, trace=False, **spmd_kwargs):
    from concourse.bass_utils import run_bass_kernel_spmd

    plan, in_maps, tq = _prepare(inputs)
    nc = _build_nc(plan["ng"], plan["npd"], plan["spd"], plan["tps"], tq)
    nc.compile()
    res = run_bass_kernel_spmd(nc, in_maps, core_ids=list(range(C)), trace=trace,
                               **spmd_kwargs)
    outs = [r["out"] for r in res.results]
    return _assemble(plan, outs, inputs["x"].shape[0]), res


def kernel(**inputs):
    out, _ = _run(inputs)
    return out



# revision 7
# speedup vs baseline: 1.5888x; 1.5888x over previous
"""GATNet (2x GATConv + MLP head + log_softmax) on 8 Trainium2 NeuronCores.

Strategy (dst-partitioned message passing, v2):
  - Host assigns destination nodes to 8 devices x SPD slots (32 nodes/slot),
    balancing in-edge counts so every slot has exactly TPS 128-edge tiles.
    Every device runs an identical program; per-device data differ.
  - Layer tables T = [h | a_s] live in DRAM as 512B node-PAIR rows (bf16).
    Per chunk (4 slots = 128 dst nodes, ch=4*tps edge tiles) the kernel
    dma_gathers source pairs, builds attention edge-wise, and aggregates
    messages per slot with one-hot matmuls into a single 132-wide PSUM
    accumulator ([even-msg 64 | exp 4 | odd-msg 64]) at 4 partition offsets.
  - Layer-2 node table rows are produced inside layer 1's epilogue (each
    device computes rows only for its own nodes) and shared via 4 sliced
    AllGathers that overlap the remaining layer-1 chunks.  The global node
    numbering is slice-major so every collective output range is contiguous.
  - a_e (both layers) is computed once in layer 1 and parked in SBUF; a_d
    tables also stay resident in SBUF.  log_softmax's Ln runs once at the
    end (avoids per-chunk activation-table reloads); the final output is
    written with a single DMA.

Numerics: exp() is computed directly (denominator-deferred softmax; alpha is
O(1) so segment-max subtraction is unnecessary).  h, a_s, one-hots, exp
weights and messages travel as bf16; PSUM accumulation, node-level math and
the head stay fp32.
"""

import numpy as np

# model constants (fixed by the problem)
IN = 128
HID = 16
OUT = 40
H = 4
ED = 16
HC = 64  # HID * H
NEG = 0.2
EPS = 1e-16

C = 8          # NeuronCores
NSLOT = 32     # nodes per slot (= one-hot width, PSUM col-block)
KSLICE = 4     # collective slices for the layer-2 table


def _bf16():
    import concourse.mybir as mybir
    return mybir.dt.np(mybir.dt.bfloat16)


# ----------------------------------------------------------------------------
# host-side plan: balance nodes into (device, slot) bins, lay out edge shards
# ----------------------------------------------------------------------------

def _build_plan(src, dst, n_nodes):
    """Returns a dict with the full sharding plan. src/dst include self-loops."""
    import heapq

    deg = np.bincount(dst, minlength=n_nodes).astype(np.int64)
    e_tot = src.shape[0]

    def try_pack(nbins, cap_e):
        # LPT: heaviest nodes first into least-loaded feasible bin
        order = np.argsort(-deg, kind="stable")
        loads = [(0, b) for b in range(nbins)]
        heapq.heapify(loads)
        bin_of_t = np.empty(n_nodes, np.int64)
        bin_cnt = np.zeros(nbins, np.int64)
        bin_load = np.zeros(nbins, np.int64)
        for nd in order:
            d = int(deg[nd])
            spill = []
            placed = False
            while loads:
                l, b = heapq.heappop(loads)
                if bin_cnt[b] < NSLOT and bin_load[b] + d <= cap_e:
                    bin_of_t[nd] = b
                    bin_cnt[b] += 1
                    bin_load[b] += d
                    heapq.heappush(loads, (bin_load[b], b))
                    placed = True
                    break
                elif bin_cnt[b] < NSLOT:
                    spill.append((l, b))
                # full bins are dropped
            for it in spill:
                heapq.heappush(loads, it)
            if not placed:
                return None
        return bin_of_t

    # search (slots-per-device, even tiles-per-slot) minimizing total tiles
    spd_min = 4 * int(np.ceil(n_nodes / (C * NSLOT * 4)))  # node-capacity floor
    best = None  # (tq, spd, tps, bin_of)
    for spd_try in range(spd_min, spd_min + 65, 4):
        nbins = C * spd_try
        tps_lo = int(np.ceil(e_tot / nbins / 128.0))
        tps_lo += tps_lo % 2  # ch = 4*tps must be a multiple of 8
        for tps_try in (tps_lo, tps_lo + 2):
            if best is not None and spd_try * tps_try >= best[0]:
                continue
            got = try_pack(nbins, tps_try * 128)
            if got is not None:
                best = (spd_try * tps_try, spd_try, tps_try, got)
                break
        if best is not None and (spd_try + 4) * 2 >= best[0]:
            break
    assert best is not None, "balancer failed"
    _, spd, tps, bin_of = best

    nbins = C * spd
    npd = spd * NSLOT
    ng = C * npd
    assert ng // 2 <= 32767, "pair index must fit int16"

    # position of each node within its bin
    pos_of = np.zeros(n_nodes, np.int64)
    fill = np.zeros(nbins, np.int64)
    for nd in range(n_nodes):
        b = bin_of[nd]
        pos_of[nd] = fill[b]
        fill[b] += 1

    # slice-major global numbering: quads are split into KSLICE slices; the
    # global table is [slice][device][quad-in-slice][128].  This makes every
    # sliced AllGather output range contiguous.
    qpd = spd // 4
    qs = [qpd // KSLICE + (1 if i < qpd % KSLICE else 0) for i in range(KSLICE)]
    q_bounds = np.zeros(KSLICE + 1, np.int64)
    np.cumsum(qs, out=q_bounds[1:])
    slice_of_q = np.searchsorted(q_bounds, np.arange(qpd), side="right") - 1

    dev_of_bin = np.arange(nbins) // spd
    s_of_bin = np.arange(nbins) % spd
    q_of_bin = s_of_bin // 4
    j_of_bin = s_of_bin % 4
    k_of_bin = slice_of_q[q_of_bin]
    node_base_k = C * 128 * q_bounds  # global node offset of each slice
    g_of_bin = (node_base_k[k_of_bin]
                + dev_of_bin * (np.array(qs)[k_of_bin] * 128)
                + (q_of_bin - q_bounds[k_of_bin]) * 128
                + j_of_bin * 32)
    node2g = (g_of_bin[bin_of] + pos_of).astype(np.int64)

    # edges sorted by destination bin; rank within bin
    ebin = bin_of[dst]
    order = np.argsort(ebin, kind="stable")
    counts = np.bincount(ebin, minlength=nbins)
    cap = tps * 128
    assert counts.max() <= cap
    starts = np.zeros(nbins + 1, np.int64)
    np.cumsum(counts, out=starts[1:])
    rank = np.arange(e_tot, dtype=np.int64) - starts[ebin[order]]
    canvas = np.full((nbins, cap), -1, np.int64)       # edge id or -1 pad
    canvas[ebin[order], rank] = order

    return dict(
        spd=spd, tps=tps, npd=npd, ng=ng, nbins=nbins, qpd=qpd,
        q_bounds=q_bounds, bin_of=bin_of, pos_of=pos_of, node2g=node2g,
        g_of_bin=g_of_bin, canvas=canvas,
    )


def _host_arrays(plan, x, src, dst, edge_attr, mean_attr, n_nodes):
    """Per-core input arrays."""
    bf16 = _bf16()
    spd, tps, npd, ng = plan["spd"], plan["tps"], plan["npd"], plan["ng"]
    node2g, pos_of, canvas = plan["node2g"], plan["pos_of"], plan["canvas"]
    tq = spd * tps                       # 128-edge tiles per device
    ch = 4 * tps
    e0 = edge_attr.shape[0]

    # permuted node features, transposed: xT [IN, ng] (bf16)
    xg = np.zeros((ng, IN), np.float32)
    xg[node2g] = np.asarray(x, np.float32)
    xT = np.ascontiguousarray(xg.T.astype(bf16))

    per_core = []
    for d in range(C):
        cv = canvas[d * spd:(d + 1) * spd].reshape(tq, 128)  # [tile, lane]
        valid = cv >= 0
        eid = np.where(valid, cv, 0)
        srcg = np.where(valid, node2g[src[eid]], 0)
        # pair index (int16, wrapped in 16 partitions: idx i at [i%16, i//16])
        srcp = (srcg >> 1).astype(np.int16).reshape(tq * 128)   # i = t*128+p
        srcp_w = np.tile(srcp.reshape(-1, 16).T, (8, 1))        # [128, tq*8]
        par = (srcg & 1).astype(np.float32).T                   # [128, tq]
        par4 = np.repeat(par.astype(bf16)[:, :, None], 4, axis=2
                         ).reshape(128, tq * 4).view(np.int16)  # [128, tq*4]
        drel = np.where(valid, pos_of[dst[eid]].astype(np.float32), -1.0)
        drelb = drel.T.astype(bf16).view(np.int16)              # [128, tq]
        # comb: per-chunk blocks of [idx(8ch) | par4(4ch) | drel(ch)] int16
        nq = tq // ch
        comb = np.empty((128, tq * 13), np.int16)
        cw = comb.reshape(128, nq, 13 * ch)
        cw[:, :, 0:8 * ch] = srcp_w.reshape(128, nq, 8 * ch)
        cw[:, :, 8 * ch:12 * ch] = par4.reshape(128, nq, 4 * ch)
        cw[:, :, 12 * ch:13 * ch] = drelb.reshape(128, nq, ch)
        ea = np.zeros((tq, 128, ED), np.float32)
        sel = valid & (eid < e0)
        ea[sel] = edge_attr[eid[sel]]
        loop_sel = valid & (eid >= e0)
        ea[loop_sel] = mean_attr
        # eaTg[(tt)*16 + r, g*128 + p]: groups of 8 tiles (bf16)
        eaTg = np.ascontiguousarray(
            ea.reshape(tq // 8, 8, 128, ED).transpose(1, 3, 0, 2)
            .reshape(128, (tq // 8) * 128).astype(bf16))
        # local columns: global index of every (slot, pos) of this device
        g_loc = (plan["g_of_bin"][d * spd:(d + 1) * spd, None]
                 + np.arange(NSLOT)[None, :]).reshape(npd)
        per_core.append(dict(
            comb=np.ascontiguousarray(comb), eaTg=eaTg,
            xTloc=np.ascontiguousarray(xT[:, g_loc]),
        ))
    return per_core, xT, tq


def _fold_weights(W1, att_s1, att_d1, We1, att_e1, b1,
                  W2, att_s2, att_d2, We2, att_e2, b2,
                  lw1, lb1, lw2, lb2):
    bf16 = _bf16()

    def head_fold(att):  # [H, HID] -> [HC, H] block diag columns
        A = np.zeros((HC, H), np.float32)
        for h in range(H):
            A[h * HID:(h + 1) * HID, h] = att[h]
        return A

    W1aug = np.concatenate([W1, W1 @ head_fold(att_s1), W1 @ head_fold(att_d1)], 1)
    W2aug = np.concatenate([W2, W2 @ head_fold(att_s2), W2 @ head_fold(att_d2)], 1)
    Ve = np.zeros((ED, 8), np.float32)
    for h in range(H):
        Ve[:, h] = We1[:, h * HID:(h + 1) * HID] @ att_e1[h]
        Ve[:, 4 + h] = We2[:, h * HID:(h + 1) * HID] @ att_e2[h]
    VeBD = np.zeros((128, 64), np.float32)
    for j in range(8):
        VeBD[ED * j:ED * (j + 1), 8 * j:8 * (j + 1)] = Ve
    LW = (lw1 @ lw2).astype(np.float32)
    lb2p = (lb1 @ lw2 + lb2).astype(np.float32)
    return (W1aug.astype(bf16), W2aug.astype(bf16), VeBD.astype(bf16),
            LW.astype(bf16), lb2p, b1.astype(np.float32), b2.astype(np.float32))


# ----------------------------------------------------------------------------
# the bass program (identical for all cores)
# ----------------------------------------------------------------------------

def _build_nc(ng, npd, spd, tps, tq, q_bounds):
    import concourse.bass as bass
    import concourse.mybir as mybir
    import concourse.tile as tile
    from concourse import bacc
    from contextlib import ExitStack

    F32 = mybir.dt.float32
    BF16 = mybir.dt.bfloat16
    I16 = mybir.dt.int16
    ALU = mybir.AluOpType
    ACT = mybir.ActivationFunctionType

    ch = 4 * tps          # tiles per chunk (one quad = 4 slots)
    qpd = spd // 4        # chunks per device per layer
    nt = ng // 128        # node tiles (table build)
    jpd = npd // 128      # local 128-node groups (== qpd)
    ngr = ch // 8         # eaTg groups per chunk
    q_bounds = [int(v) for v in q_bounds]

    nc = bacc.Bacc(None, target_bir_lowering=False)

    # kernel IO
    t_xT = nc.dram_tensor("xT", [128, ng], BF16, kind="ExternalInput")
    t_xTl = nc.dram_tensor("xTloc", [128, npd], BF16, kind="ExternalInput")
    t_eaTg = nc.dram_tensor("eaTg", [128, (tq // 8) * 128], BF16, kind="ExternalInput")
    t_comb = nc.dram_tensor("comb", [128, tq * 13], I16, kind="ExternalInput")
    t_W1 = nc.dram_tensor("W1aug", [128, 72], BF16, kind="ExternalInput")
    t_W2 = nc.dram_tensor("W2aug", [64, 72], BF16, kind="ExternalInput")
    t_VeBD = nc.dram_tensor("VeBD", [128, 64], BF16, kind="ExternalInput")
    t_LW = nc.dram_tensor("LW", [64, OUT], BF16, kind="ExternalInput")
    t_cst = nc.dram_tensor("cst", [1, 256], F32, kind="ExternalInput")
    # cst row: [b1(64) | b2(64) | lb2p(40) | iota32(32) | pad]
    t_mask = nc.dram_tensor("bdmask", [128, 16], BF16, kind="ExternalInput")
    t_I = nc.dram_tensor("ident", [128, 128], F32, kind="ExternalInput")
    t_out = nc.dram_tensor("out", [npd, OUT], F32, kind="ExternalOutput")

    # internal DRAM.  Node tables: 512B per node pair (bf16).
    d_T1 = nc.dram_tensor("T1", [ng // 2, 256], BF16)
    d_T2 = nc.dram_tensor("T2", [ng // 2, 256], BF16, addr_space="Shared")
    d_T2loc = [nc.dram_tensor(f"T2loc{k}", [64 * (q_bounds[k + 1] - q_bounds[k]), 256], BF16)
               for k in range(KSLICE)]

    def rows(tbl):  # [ng, 128] bf16 row view of the pair table
        return tbl.ap().rearrange("m (two d) -> (m two) d", two=2)

    with tile.TileContext(nc) as tc, ExitStack() as top:
        cp = top.enter_context(tc.tile_pool(name="consts", bufs=1))
        pers = top.enter_context(tc.tile_pool(name="persist", bufs=1))

        W1sb = cp.tile([128, 72], BF16)
        W2sb = cp.tile([64, 72], BF16)
        VeBD = cp.tile([128, 64], BF16)
        LWsb = cp.tile([64, OUT], BF16)
        Isb = cp.tile([128, 128], F32)
        maskb = cp.tile([128, 16], BF16)
        b1bc = cp.tile([128, 64], F32)
        b2bc = cp.tile([128, 64], F32)
        lbbc = cp.tile([128, OUT], F32)
        iota = cp.tile([128, NSLOT], F32)
        iotab = cp.tile([128, NSLOT], BF16)
        Ib16 = cp.tile([128, 128], BF16)
        nc.sync.dma_start(W1sb[:], t_W1[:, :])
        nc.sync.dma_start(W2sb[:], t_W2[:, :])
        nc.sync.dma_start(VeBD[:], t_VeBD[:, :])
        nc.sync.dma_start(LWsb[:], t_LW[:, :])
        nc.sync.dma_start(Isb[:], t_I[:, :])
        nc.sync.dma_start(maskb[:], t_mask[:, :])
        nc.sync.dma_start(b1bc[:], t_cst[:, 0:64].partition_broadcast(128))
        nc.sync.dma_start(b2bc[:], t_cst[:, 64:128].partition_broadcast(128))
        nc.sync.dma_start(lbbc[:], t_cst[:, 128:128 + OUT].partition_broadcast(128))
        nc.sync.dma_start(iota[:], t_cst[:, 168:168 + NSLOT].partition_broadcast(128))
        nc.vector.tensor_copy(out=Ib16[:], in_=Isb[:])
        nc.vector.tensor_copy(out=iotab[:], in_=iota[:])

        # persistent SBUF state
        ae2sb = pers.tile([128, tq, 4], BF16)       # layer-2 a_e per edge
        ad1sb = pers.tile([128, jpd, 4], BF16)      # layer-1 a_d per local node
        ad2sb = pers.tile([128, jpd, 4], BF16)      # layer-2 a_d per local node
        zall = pers.tile([128, qpd, OUT], F32)      # head logits (shifted)
        smsb = pers.tile([128, qpd], F32)           # softmax sums

        # ---------------- phase A1: T1 = [x@W1 | a_s1]; local a_d1 ----------
        with ExitStack() as ph:
            ap = ph.enter_context(tc.tile_pool(name="pa_sb", bufs=3))
            app = ph.enter_context(tc.tile_pool(name="pa_ps", bufs=2, space="PSUM"))
            for i0 in range(0, nt, 8):
                bs = min(8, nt - i0)
                xt = ap.tile([128, 8 * 128], BF16, tag="xt")
                nc.sync.dma_start(xt[:, 0:128 * bs], t_xT[:, 128 * i0:128 * (i0 + bs)])
                ps0 = app.tile([128, 4, 72], F32, tag="ps0")
                ps1 = app.tile([128, 4, 72], F32, tag="ps1")
                for c in range(bs):
                    pst = ps0 if c < 4 else ps1
                    nc.tensor.matmul(pst[:, c % 4, :], xt[:, 128 * c:128 * (c + 1)],
                                     W1sb[:], start=True, stop=True)
                hsb = ap.tile([128, 8, 128], BF16, tag="hsb")
                nc.vector.tensor_copy(out=hsb[:, 0:4, 0:68], in_=ps0[:, :, 0:68])
                if bs > 4:
                    nc.vector.tensor_copy(out=hsb[:, 4:bs, 0:68],
                                          in_=ps1[:, 0:bs - 4, 0:68])
                nc.gpsimd.dma_start(
                    rows(d_T1)[128 * i0:128 * (i0 + bs), :].rearrange(
                        "(c r) d -> r c d", c=bs),
                    hsb[:, 0:bs, :])
            for jj0 in range(0, jpd, 8):
                bs = min(8, jpd - jj0)
                xt = ap.tile([128, 8 * 128], BF16, tag="xt")
                nc.sync.dma_start(xt[:, 0:128 * bs], t_xTl[:, 128 * jj0:128 * (jj0 + bs)])
                psa = app.tile([128, 32], F32, tag="psa")
                for c in range(bs):
                    nc.tensor.matmul(psa[:, 4 * c:4 * (c + 1)],
                                     xt[:, 128 * c:128 * (c + 1)],
                                     W1sb[:, 68:72], start=True, stop=True)
                nc.vector.tensor_copy(
                    out=ad1sb[:, jj0:jj0 + bs, :],
                    in_=psa[:, 0:4 * bs].rearrange("p (c v) -> p c v", v=4))

        # ---------------- edge phase (shared for both layers) ----------------
        def edge_layer(layer, tbl, adsb):
            with ExitStack() as ph:
                ip = ph.enter_context(tc.tile_pool(name=f"l{layer}_idx", bufs=3))
                gp = ph.enter_context(tc.tile_pool(name=f"l{layer}_g", bufs=3))
                sp = ph.enter_context(tc.tile_pool(name=f"l{layer}_s", bufs=2))
                mp = ph.enter_context(tc.tile_pool(name=f"l{layer}_m", bufs=2))
                ep = ph.enter_context(tc.tile_pool(name=f"l{layer}_e", bufs=2))
                pp = ph.enter_context(tc.tile_pool(name=f"l{layer}_ps", bufs=2, space="PSUM"))
                p1 = ph.enter_context(tc.tile_pool(name=f"l{layer}_p1", bufs=1, space="PSUM"))

                for q in range(qpd):
                    c0 = ch * q
                    # --- per-chunk int16 block: [idx(8ch) | par4(4ch) | drel(ch)]
                    comb = ip.tile([128, 13 * ch], I16, tag="comb")
                    nc.sync.dma_start(comb[:], t_comb[:, 13 * c0:13 * (c0 + ch)])
                    idxv = comb[:, 0:8 * ch]
                    par4 = comb[:, 8 * ch:12 * ch].bitcast(BF16)   # [128, 4ch]
                    drelb = comb[:, 12 * ch:13 * ch].bitcast(BF16)  # [128, ch]

                    g2 = gp.tile([128, ch, 256], BF16, tag="g2")
                    nc.gpsimd.dma_gather(
                        out_ap=g2[:], in_ap=tbl[:, :], idxs_ap=idxv,
                        num_idxs=ch * 128, num_idxs_reg=ch * 128, elem_size=256,
                        single_packet=False)

                    # --- one-hot S, batch-major: [128, tps(b), 4(j), NSLOT]
                    S = sp.tile([128, tps, 4, NSLOT], BF16, tag="S")
                    nc.vector.tensor_tensor(
                        out=S[:],
                        in0=drelb.rearrange("p (j b) -> p b j", b=tps)
                            .unsqueeze(3).to_broadcast([128, tps, 4, NSLOT]),
                        in1=iotab[:].unsqueeze(1).unsqueeze(1)
                            .to_broadcast([128, tps, 4, NSLOT]),
                        op=ALU.is_equal)

                    # --- a_d expansion: S^T via PE, block-diag a_d matmul
                    bd = ip.tile([128, 16], BF16, tag="bd")
                    nc.vector.tensor_tensor(
                        out=bd[:],
                        in0=adsb[:, q, :].unsqueeze(1).to_broadcast([128, 4, 4]),
                        in1=maskb[:].rearrange("p (j v) -> p j v", v=4),
                        op=ALU.mult)
                    alad = p1.tile([128, tps * 16], F32, tag="alad")
                    for b0 in range(0, tps, 2):
                        stp = p1.tile([128, 256], BF16, tag="stp")
                        for b in (b0, b0 + 1):
                            nc.tensor.transpose(
                                stp[:, 128 * (b - b0):128 * (b - b0 + 1)],
                                S[:, b, :, :].rearrange("p a w -> p (a w)"), Ib16[:])
                        sts = sp.tile([128, 256], BF16, tag="sts")
                        nc.scalar.activation(sts[:], stp[:], ACT.Copy)
                        for b in (b0, b0 + 1):
                            nc.tensor.matmul(alad[:, 16 * b:16 * (b + 1)],
                                             sts[:, 128 * (b - b0):128 * (b - b0 + 1)],
                                             bd[:], start=True, stop=True)
                    aladb = ep.tile([128, tps * 16], BF16, tag="aladb")
                    nc.scalar.activation(aladb[:], alad[:], ACT.Copy)

                    # --- a_e
                    if layer == 1:
                        eac = ip.tile([128, 128 * ngr], BF16, tag="eac")
                        nc.sync.dma_start(eac[:], t_eaTg[:, 128 * ngr * q:128 * ngr * (q + 1)])
                        aeT = p1.tile([64, 128 * ngr], F32, tag="aeT")
                        for gi in range(ngr):
                            nc.tensor.matmul(aeT[:, 128 * gi:128 * (gi + 1)], VeBD[:],
                                             eac[:, 128 * gi:128 * (gi + 1)],
                                             start=True, stop=True)
                        aeTs = ep.tile([64, 128 * ngr], BF16, tag="aeTs")
                        nc.scalar.activation(aeTs[:], aeT[:], ACT.Copy)
                        aeps = p1.tile([128, 64 * ngr], F32, tag="aeps")
                        for gi in range(ngr):
                            nc.tensor.matmul(aeps[:, 64 * gi:64 * (gi + 1)],
                                             aeTs[:, 128 * gi:128 * (gi + 1)],
                                             Ib16[0:64, 0:64], start=True, stop=True)
                        aesb = ep.tile([128, ngr, 8, 8], BF16, tag="aesb")
                        nc.scalar.activation(
                            aesb[:].rearrange("p a b c -> p (a b c)"),
                            aeps[:], ACT.Copy)
                        nc.vector.tensor_copy(
                            out=ae2sb[:, c0:c0 + ch, :],
                            in_=aesb[:, :, :, 4:8])

                    # --- alpha = a_s[src](parity-sel) + a_d[dst] + a_e
                    al = mp.tile([128, ch, 4], BF16, tag="al")
                    nc.vector.tensor_tensor(out=al[:], in0=g2[:, :, 192:196],
                                            in1=g2[:, :, 64:68], op=ALU.subtract)
                    nc.vector.tensor_tensor(
                        out=al[:], in0=al[:],
                        in1=par4.rearrange("p (t v) -> p t v", v=4), op=ALU.mult)
                    nc.vector.tensor_tensor(out=al[:], in0=al[:], in1=g2[:, :, 64:68],
                                            op=ALU.add)
                    if layer == 1:
                        nc.vector.tensor_tensor(out=al[:], in0=al[:],
                                                in1=aesb[:, :, :, 0:4], op=ALU.add)
                    else:
                        nc.vector.tensor_tensor(out=al[:], in0=al[:],
                                                in1=ae2sb[:, c0:c0 + ch, :], op=ALU.add)
                    # += a_d (tile (j, b) lives at aladb[:, 16b + 4j : +4])
                    nc.vector.tensor_tensor(
                        out=al[:], in0=al[:],
                        in1=aladb[:].rearrange("p (b j v) -> p j b v", j=4, v=4),
                        op=ALU.add)
                    # leaky relu + exp (bf16)
                    lk = mp.tile([128, ch, 4], BF16, tag="lk")
                    nc.vector.tensor_scalar_mul(lk[:], al[:], NEG)
                    nc.vector.tensor_tensor(out=lk[:], in0=al[:], in1=lk[:], op=ALU.max)
                    exb = mp.tile([128, ch, 4], BF16, tag="exb")
                    nc.scalar.activation(exb[:], lk[:], ACT.Exp)
                    # parity-split exp weights + x2 expansion (packed-dim trick)
                    exo = mp.tile([128, ch, 4], BF16, tag="exo")
                    nc.vector.tensor_tensor(
                        out=exo[:], in0=exb[:],
                        in1=par4.rearrange("p (t v) -> p t v", v=4), op=ALU.mult)
                    exe = mp.tile([128, ch, 4], BF16, tag="exe")
                    nc.vector.tensor_tensor(out=exe[:], in0=exb[:], in1=exo[:],
                                            op=ALU.subtract)
                    exo2 = mp.tile([128, ch, 4, 2], BF16, tag="exo2")
                    nc.scalar.activation(
                        exo2[:],
                        exo[:].unsqueeze(3).to_broadcast([128, ch, 4, 2]), ACT.Copy)
                    exe2 = mp.tile([128, ch, 4, 2], BF16, tag="exe2")
                    nc.scalar.activation(
                        exe2[:],
                        exe[:].unsqueeze(3).to_broadcast([128, ch, 4, 2]), ACT.Copy)

                    # --- messages: [even*exe (64) | ex (4) | odd*exo (64)]
                    msg = mp.tile([128, ch, 132], BF16, tag="msg")
                    nc.vector.tensor_tensor(
                        out=msg[:, :, 0:64].rearrange("p t (h c e) -> p t h c e",
                                                      h=H, e=2),
                        in0=g2[:, :, 0:64].rearrange("p t (h c e) -> p t h c e",
                                                     h=H, e=2),
                        in1=exe2[:].unsqueeze(3).to_broadcast([128, ch, 4, 8, 2]),
                        op=ALU.mult)
                    nc.vector.tensor_tensor(
                        out=msg[:, :, 68:132].rearrange("p t (h c e) -> p t h c e",
                                                        h=H, e=2),
                        in0=g2[:, :, 128:192].rearrange("p t (h c e) -> p t h c e",
                                                        h=H, e=2),
                        in1=exo2[:].unsqueeze(3).to_broadcast([128, ch, 4, 8, 2]),
                        op=ALU.mult)
                    nc.vector.tensor_copy(out=msg[:, :, 64:68], in_=exb[:])

                    # --- aggregate per slot: one 132-wide PSUM, 4 row blocks
                    U = pp.tile([128, 132], F32, tag="U")
                    for j in range(4):
                        for tt in range(tps):
                            t = tps * j + tt
                            nc.tensor.matmul(U[32 * j:32 * (j + 1), :],
                                             S[:, tt, j, :], msg[:, t, :],
                                             start=(tt == 0), stop=(tt == tps - 1),
                                             tile_position=(0, 32 * j))

                    # --- epilogue: out = (UE+UO)/(den+eps) + bias, relu
                    Usb = ep.tile([128, 64], F32, tag="Usb")
                    nc.vector.tensor_copy(out=Usb[:], in_=U[:, 0:64])
                    nc.vector.tensor_tensor(out=Usb[:], in0=Usb[:],
                                            in1=U[:, 68:132], op=ALU.add)
                    rec = ep.tile([128, 4], F32, tag="rec")
                    nc.vector.tensor_scalar_add(rec[:], U[:, 64:68], EPS)
                    nc.vector.reciprocal(rec[:], rec[:])
                    outq = ep.tile([128, 64], F32, tag="outq")
                    nc.vector.tensor_tensor(
                        out=outq[:].rearrange("p (h c) -> p h c", h=H),
                        in0=Usb[:].rearrange("p (h c) -> p h c", h=H),
                        in1=rec[:].unsqueeze(2).to_broadcast([128, H, HID]),
                        op=ALU.mult)
                    bias = b1bc if layer == 1 else b2bc
                    nc.vector.tensor_tensor(out=outq[:], in0=outq[:], in1=bias[:],
                                            op=ALU.add)
                    nc.vector.tensor_scalar_max(outq[:], outq[:], 0.0)

                    # transpose out_quad (both layers need it)
                    tp = p1.tile([128, 128], F32, tag="tp")
                    nc.tensor.transpose(tp[0:64, :], outq[:], Isb[:])
                    tpsb = ep.tile([64, 128], BF16, tag="tpsb")
                    nc.scalar.activation(tpsb[:], tp[0:64, :], ACT.Copy)

                    if layer == 1:
                        # layer-2 table rows + a_d2 for this quad's 128 nodes
                        t2ps = p1.tile([128, 72], F32, tag="t2ps")
                        nc.tensor.matmul(t2ps[:], tpsb[:], W2sb[:], start=True, stop=True)
                        t2sb = ep.tile([128, 128], BF16, tag="t2sb")
                        nc.scalar.activation(t2sb[:, 0:68], t2ps[:, 0:68], ACT.Copy)
                        nc.vector.tensor_copy(out=ad2sb[:, q, :], in_=t2ps[:, 68:72])
                        k = next(i for i in range(KSLICE) if q < q_bounds[i + 1])
                        r0 = 64 * (q - q_bounds[k])
                        nc.gpsimd.dma_start(
                            d_T2loc[k].ap()[r0:r0 + 64, :].rearrange(
                                "r (two d) -> (r two) d", two=2),
                            t2sb[:])
                        if q == q_bounds[k + 1] - 1:
                            nc.gpsimd.collective_compute(
                                "AllGather", mybir.AluOpType.bypass,
                                replica_groups=[list(range(C))],
                                ins=[d_T2loc[k].ap()],
                                outs=[d_T2.ap()[C * 64 * q_bounds[k]:
                                                C * 64 * q_bounds[k + 1], :]],
                            )
                    else:
                        # head: logits into zall; exp-sum into smsb
                        lg = p1.tile([128, OUT], F32, tag="lg")
                        nc.tensor.matmul(lg[:], tpsb[:], LWsb[:], start=True, stop=True)
                        nc.vector.tensor_tensor(out=zall[:, q, :], in0=lg[:],
                                                in1=lbbc[:], op=ALU.add)
                        mx = ep.tile([128, 1], F32, tag="mx")
                        nc.vector.reduce_max(mx[:], zall[:, q, :],
                                             axis=mybir.AxisListType.X)
                        nc.vector.tensor_scalar(out=zall[:, q, :], in0=zall[:, q, :],
                                                scalar1=mx[:], scalar2=None,
                                                op0=ALU.subtract)
                        ez = ep.tile([128, OUT], BF16, tag="ez")
                        nc.scalar.activation(ez[:], zall[:, q, :], ACT.Exp,
                                             accum_out=smsb[:, q:q + 1])

        edge_layer(1, d_T1, ad1sb)
        edge_layer(2, d_T2, ad2sb)

        # ---------------- tail: log_softmax finish + single output DMA -------
        with ExitStack() as ph:
            tpool = ph.enter_context(tc.tile_pool(name="tail", bufs=1))
            lnall = tpool.tile([128, qpd], F32)
            nc.scalar.activation(lnall[:], smsb[:], ACT.Ln)
            nc.vector.tensor_tensor(
                out=zall[:],
                in0=zall[:],
                in1=lnall[:].unsqueeze(2).to_broadcast([128, qpd, OUT]),
                op=ALU.subtract)
            nc.sync.dma_start(
                t_out.ap().rearrange("(q r) d -> r q d", r=128),
                zall[:])

    return nc


# ----------------------------------------------------------------------------
# public entry
# ----------------------------------------------------------------------------

def _prepare(inputs):
    x = np.asarray(inputs["x"], np.float32)
    ei = np.asarray(inputs["edge_index"], np.int64)
    ea = np.asarray(inputs["edge_attr"], np.float32)
    n = x.shape[0]
    loop = np.arange(n, dtype=np.int64)
    src = np.concatenate([ei[0], loop])
    dst = np.concatenate([ei[1], loop])
    mean_attr = ea.mean(axis=0)

    plan = _build_plan(src, dst, n)
    per_core, xT, tq = _host_arrays(plan, x, src, dst, ea, mean_attr, n)

    W1aug, W2aug, VeBD, LW, lb2p, b1, b2 = _fold_weights(
        np.asarray(inputs["W1"], np.float32), np.asarray(inputs["att_src1"], np.float32),
        np.asarray(inputs["att_dst1"], np.float32), np.asarray(inputs["We1"], np.float32),
        np.asarray(inputs["att_e1"], np.float32), np.asarray(inputs["b1"], np.float32),
        np.asarray(inputs["W2"], np.float32), np.asarray(inputs["att_src2"], np.float32),
        np.asarray(inputs["att_dst2"], np.float32), np.asarray(inputs["We2"], np.float32),
        np.asarray(inputs["att_e2"], np.float32), np.asarray(inputs["b2"], np.float32),
        np.asarray(inputs["lw1"], np.float32), np.asarray(inputs["lb1"], np.float32),
        np.asarray(inputs["lw2"], np.float32), np.asarray(inputs["lb2"], np.float32))

    bf16 = _bf16()
    cst = np.zeros((1, 256), np.float32)
    cst[0, 0:64] = b1
    cst[0, 64:128] = b2
    cst[0, 128:128 + OUT] = lb2p
    cst[0, 168:168 + NSLOT] = np.arange(NSLOT, dtype=np.float32)
    ident = np.eye(128, dtype=np.float32)
    bdmask = np.zeros((128, 16), np.float32)
    for j in range(4):
        bdmask[32 * j:32 * (j + 1), 4 * j:4 * (j + 1)] = 1.0
    bdmask = bdmask.astype(bf16)

    in_maps = []
    for d in range(C):
        pc = per_core[d]
        in_maps.append({
            "xT": xT, "xTloc": pc["xTloc"], "eaTg": pc["eaTg"], "comb": pc["comb"],
            "W1aug": W1aug, "W2aug": W2aug, "VeBD": VeBD, "LW": LW, "cst": cst,
            "ident": ident, "bdmask": bdmask,
        })
    return plan, in_maps, tq


def _assemble(plan, outs, n):
    bin_of, pos_of, spd = plan["bin_of"], plan["pos_of"], plan["spd"]
    dev = bin_of // spd
    loc = (bin_of % spd) * NSLOT + pos_of
    stacked = np.stack([np.asarray(o, np.float32) for o in outs])  # [C, npd, OUT]
    return stacked[dev[:n], loc[:n]]


def _run(inputs, trace=False, **spmd_kwargs):
    from concourse.bass_utils import run_bass_kernel_spmd

    plan, in_maps, tq = _prepare(inputs)
    nc = _build_nc(plan["ng"], plan["npd"], plan["spd"], plan["tps"], tq,
                   plan["q_bounds"])
    nc.compile()
    res = run_bass_kernel_spmd(nc, in_maps, core_ids=list(range(C)), trace=trace,
                               **spmd_kwargs)
    outs = [r["out"] for r in res.results]
    return _assemble(plan, outs, inputs["x"].shape[0]), res


def kernel(**inputs):
    out, _ = _run(inputs)
    return out


# revision 11
# speedup vs baseline: 2.0549x; 1.2933x over previous
"""GATNet (2x GATConv + MLP head + log_softmax) on 8 Trainium2 NeuronCores.

Strategy (dst-partitioned message passing, v3):
  - Host assigns destination nodes to 8 devices x SPD slots (32 nodes/slot),
    balancing in-edge counts so every slot has exactly TPS 128-edge tiles.
    Every device runs an identical program; per-device data differ.
  - Layer tables T = [h(64) | a_s(4) | pad] are single 256B bf16 rows, one per
    node, [ng, 128].  int16 gather indices only reach 32767, so each slot's
    edges are sorted low-half sources first and every chunk runs TWO
    dma_gathers: A over tiles [0, nA) against table rows [0, ng/2), B over
    tiles [nB0, tps) against rows [ng/2, ng).  Only ~(nA+tps-nB0)/tps of the
    pair-gather bytes move, which is what the gather costs on the Pool queue.
  - Per chunk (4 slots = 128 dst nodes) attention is built edge-wise (bf16),
    low/high exp weights are region-masked, and messages aggregate into one
    68-wide PSUM accumulator at 4 partition offsets via one-hot matmuls.
  - Layer-2 table rows are produced inside layer 1's epilogue (each device
    computes rows only for its own nodes = device-major global numbering) and
    shared with a single AllGather between the layers.
  - a_e for both layers is computed once in layer 1 and parked in SBUF; a_d
    tables also stay resident in SBUF.  log_softmax's Ln runs once at the end
    (avoids per-chunk activation-table reloads); output leaves in one DMA.
"""

import numpy as np

# model constants (fixed by the problem)
IN = 128
HID = 16
OUT = 40
H = 4
ED = 16
HC = 64  # HID * H
NEG = 0.2
EPS = 1e-16

C = 8          # NeuronCores
NSLOT = 32     # nodes per slot (= one-hot width, PSUM col-block)


def _bf16():
    import concourse.mybir as mybir
    return mybir.dt.np(mybir.dt.bfloat16)


# ----------------------------------------------------------------------------
# host-side plan: balance nodes into (device, slot) bins, lay out edge shards
# ----------------------------------------------------------------------------

def _build_plan(src, dst, n_nodes):
    """Returns a dict with the full sharding plan. src/dst include self-loops."""
    import heapq

    deg = np.bincount(dst, minlength=n_nodes).astype(np.int64)
    e_tot = src.shape[0]

    def try_pack(nbins, cap_e):
        # LPT: heaviest nodes first into least-loaded feasible bin
        order = np.argsort(-deg, kind="stable")
        loads = [(0, b) for b in range(nbins)]
        heapq.heapify(loads)
        bin_of_t = np.empty(n_nodes, np.int64)
        bin_cnt = np.zeros(nbins, np.int64)
        bin_load = np.zeros(nbins, np.int64)
        for nd in order:
            d = int(deg[nd])
            spill = []
            placed = False
            while loads:
                l, b = heapq.heappop(loads)
                if bin_cnt[b] < NSLOT and bin_load[b] + d <= cap_e:
                    bin_of_t[nd] = b
                    bin_cnt[b] += 1
                    bin_load[b] += d
                    heapq.heappush(loads, (bin_load[b], b))
                    placed = True
                    break
                elif bin_cnt[b] < NSLOT:
                    spill.append((l, b))
                # full bins are dropped
            for it in spill:
                heapq.heappush(loads, it)
            if not placed:
                return None
        return bin_of_t

    # search (slots-per-device, even tiles-per-slot) minimizing total tiles
    spd_min = 4 * int(np.ceil(n_nodes / (C * NSLOT * 4)))  # node-capacity floor
    best = None  # (tq, spd, tps, bin_of)
    for spd_try in range(spd_min, spd_min + 65, 4):
        nbins = C * spd_try
        tps_lo = int(np.ceil(e_tot / nbins / 128.0))
        tps_lo += tps_lo % 2  # ch = 4*tps must be a multiple of 8
        for tps_try in (tps_lo, tps_lo + 2):
            if best is not None and spd_try * tps_try >= best[0]:
                continue
            got = try_pack(nbins, tps_try * 128)
            if got is not None:
                best = (spd_try * tps_try, spd_try, tps_try, got)
                break
        if best is not None and (spd_try + 4) * 2 >= best[0]:
            break
    assert best is not None, "balancer failed"
    _, spd, tps, bin_of = best

    nbins = C * spd
    npd = spd * NSLOT
    ng = C * npd
    assert ng // 2 <= 32767, "half-table index must fit int16"

    # position of each node within its bin
    pos_of = np.zeros(n_nodes, np.int64)
    fill = np.zeros(nbins, np.int64)
    for nd in range(n_nodes):
        b = bin_of[nd]
        pos_of[nd] = fill[b]
        fill[b] += 1
    # device-major global numbering (one AllGather concat = the table order)
    node2g = ((bin_of // spd) * npd + (bin_of % spd) * NSLOT + pos_of).astype(np.int64)

    # edges per destination bin, LOW-half sources first, then high, then pad
    ebin = bin_of[dst]
    half = (node2g[src] >= ng // 2).astype(np.int64)
    order = np.lexsort((half, ebin))          # by bin, low-src first (stable)
    counts = np.bincount(ebin, minlength=nbins)
    lowcnt = np.bincount(ebin[half == 0], minlength=nbins)
    cap = tps * 128
    assert counts.max() <= cap
    starts = np.zeros(nbins + 1, np.int64)
    np.cumsum(counts, out=starts[1:])
    rank = np.arange(e_tot, dtype=np.int64) - starts[ebin[order]]
    canvas = np.full((nbins, cap), -1, np.int64)       # edge id or -1 pad
    canvas[ebin[order], rank] = order

    # static gather spans: A covers tiles [0, nA), B covers [nB0, tps)
    nA = int(np.ceil(lowcnt.max() / 128.0))
    nB0 = int(lowcnt.min() // 128)
    assert 0 <= nB0 <= nA <= tps

    return dict(
        spd=spd, tps=tps, npd=npd, ng=ng, nbins=nbins, qpd=spd // 4,
        nA=nA, nB0=nB0, bin_of=bin_of, pos_of=pos_of, node2g=node2g,
        canvas=canvas,
    )


def _host_arrays(plan, x, src, dst, edge_attr, mean_attr, n_nodes):
    """Per-core input arrays."""
    bf16 = _bf16()
    spd, tps, npd, ng = plan["spd"], plan["tps"], plan["npd"], plan["ng"]
    nA, nB0 = plan["nA"], plan["nB0"]
    nBn = tps - nB0
    node2g, pos_of, canvas = plan["node2g"], plan["pos_of"], plan["canvas"]
    tq = spd * tps                       # 128-edge tiles per device
    ch = 4 * tps
    nq = tq // ch                        # chunks
    e0 = edge_attr.shape[0]

    def wrap16(a):  # [n] i16 -> [128, n//16] gather-index wrapping
        return np.tile(a.reshape(-1, 16).T, (8, 1))

    # permuted node features, transposed: xT [IN, ng] (bf16)
    xg = np.zeros((ng, IN), np.float32)
    xg[node2g] = np.asarray(x, np.float32)
    xT = np.ascontiguousarray(xg.T.astype(bf16))

    per_core = []
    for d in range(C):
        cv = canvas[d * spd:(d + 1) * spd].reshape(tq, 128)  # [tile, lane]
        valid = cv >= 0
        eid = np.where(valid, cv, 0)
        srcg = np.where(valid, node2g[src[eid]], 0)          # [tq, 128]
        hi = (srcg >= ng // 2)
        idxA = np.where(hi, 0, srcg).astype(np.int16)
        idxB = np.where(hi, srcg - ng // 2, 0).astype(np.int16)
        m4 = np.repeat(hi.astype(bf16).T[:, :, None], 4, axis=2
                       ).reshape(128, tq * 4).view(np.int16)  # [128, tq*4]
        drel = np.where(valid, pos_of[dst[eid]].astype(np.float32), -1.0)
        drelb = drel.T.astype(bf16).view(np.int16)            # [128, tq]

        # per-chunk comb block: [idxA | idxB | m4 | drel] int16
        cwA, cwB = 32 * nA, 32 * nBn
        cw = cwA + cwB + 5 * ch
        comb = np.empty((128, nq * cw), np.int16)
        t4 = np.arange(tq).reshape(nq, 4, tps)                # chunk, j, tt
        for q in range(nq):
            blk = comb[:, q * cw:(q + 1) * cw]
            tA = t4[q, :, 0:nA].reshape(-1)                   # u = j*nA+tt
            tB = t4[q, :, nB0:tps].reshape(-1)
            blk[:, 0:cwA] = wrap16(idxA[tA].reshape(-1))
            blk[:, cwA:cwA + cwB] = wrap16(idxB[tB].reshape(-1))
            blk[:, cwA + cwB:cwA + cwB + 4 * ch] = m4[:, q * 4 * ch:(q + 1) * 4 * ch]
            blk[:, cwA + cwB + 4 * ch:] = drelb[:, q * ch:(q + 1) * ch]

        ea = np.zeros((tq, 128, ED), np.float32)
        sel = valid & (eid < e0)
        ea[sel] = edge_attr[eid[sel]]
        loop_sel = valid & (eid >= e0)
        ea[loop_sel] = mean_attr
        # eaTg[(tt)*16 + r, g*128 + p]: groups of 8 tiles (bf16)
        eaTg = np.ascontiguousarray(
            ea.reshape(tq // 8, 8, 128, ED).transpose(1, 3, 0, 2)
            .reshape(128, (tq // 8) * 128).astype(bf16))
        per_core.append(dict(
            comb=np.ascontiguousarray(comb), eaTg=eaTg,
            xTloc=np.ascontiguousarray(xT[:, d * npd:(d + 1) * npd]),
        ))
    return per_core, xT, tq


def _fold_weights(W1, att_s1, att_d1, We1, att_e1, b1,
                  W2, att_s2, att_d2, We2, att_e2, b2,
                  lw1, lb1, lw2, lb2):
    bf16 = _bf16()

    def head_fold(att):  # [H, HID] -> [HC, H] block diag columns
        A = np.zeros((HC, H), np.float32)
        for h in range(H):
            A[h * HID:(h + 1) * HID, h] = att[h]
        return A

    W1aug = np.concatenate([W1, W1 @ head_fold(att_s1), W1 @ head_fold(att_d1)], 1)
    W2aug = np.concatenate([W2, W2 @ head_fold(att_s2), W2 @ head_fold(att_d2)], 1)
    Ve = np.zeros((ED, 8), np.float32)
    for h in range(H):
        Ve[:, h] = We1[:, h * HID:(h + 1) * HID] @ att_e1[h]
        Ve[:, 4 + h] = We2[:, h * HID:(h + 1) * HID] @ att_e2[h]
    VeBD = np.zeros((128, 64), np.float32)
    for j in range(8):
        VeBD[ED * j:ED * (j + 1), 8 * j:8 * (j + 1)] = Ve
    LW = (lw1 @ lw2).astype(np.float32)
    lb2p = (lb1 @ lw2 + lb2).astype(np.float32)
    return (W1aug.astype(bf16), W2aug.astype(bf16), VeBD.astype(bf16),
            LW.astype(bf16), lb2p, b1.astype(np.float32), b2.astype(np.float32))


# ----------------------------------------------------------------------------
# the bass program (identical for all cores)
# ----------------------------------------------------------------------------

def _build_nc(ng, npd, spd, tps, tq, nA, nB0):
    import concourse.bass as bass
    import concourse.mybir as mybir
    import concourse.tile as tile
    from concourse import bacc
    from contextlib import ExitStack

    F32 = mybir.dt.float32
    BF16 = mybir.dt.bfloat16
    I16 = mybir.dt.int16
    ALU = mybir.AluOpType
    ACT = mybir.ActivationFunctionType

    ch = 4 * tps          # tiles per chunk (one quad = 4 slots)
    qpd = spd // 4        # chunks per device per layer
    nt = ng // 128        # node tiles (table build)
    jpd = npd // 128      # local 128-node groups (== qpd)
    ngr = ch // 8         # eaTg groups per chunk
    nBn = tps - nB0
    ov = nA - nB0         # mixed tiles per slot
    uA, uB = 4 * nA, 4 * nBn
    cwA, cwB = 32 * nA, 32 * nBn
    cw = cwA + cwB + 5 * ch

    nc = bacc.Bacc(None, target_bir_lowering=False)

    # kernel IO
    t_xT = nc.dram_tensor("xT", [128, ng], BF16, kind="ExternalInput")
    t_xTl = nc.dram_tensor("xTloc", [128, npd], BF16, kind="ExternalInput")
    t_eaTg = nc.dram_tensor("eaTg", [128, (tq // 8) * 128], BF16, kind="ExternalInput")
    t_comb = nc.dram_tensor("comb", [128, (tq // ch) * cw], I16, kind="ExternalInput")
    t_W1 = nc.dram_tensor("W1aug", [128, 72], BF16, kind="ExternalInput")
    t_W2 = nc.dram_tensor("W2aug", [64, 72], BF16, kind="ExternalInput")
    t_VeBD = nc.dram_tensor("VeBD", [128, 64], BF16, kind="ExternalInput")
    t_LW = nc.dram_tensor("LW", [64, OUT], BF16, kind="ExternalInput")
    t_cst = nc.dram_tensor("cst", [1, 256], F32, kind="ExternalInput")
    # cst row: [b1(64) | b2(64) | lb2p(40) | iota32(32) | pad]
    t_mask = nc.dram_tensor("bdmask", [128, 16], BF16, kind="ExternalInput")
    t_I = nc.dram_tensor("ident", [128, 128], F32, kind="ExternalInput")
    t_out = nc.dram_tensor("out", [128, qpd * OUT], F32, kind="ExternalOutput")

    # node tables: 256B bf16 row per node
    d_T1 = nc.dram_tensor("T1", [ng, 128], BF16)
    d_T2loc = nc.dram_tensor("T2loc", [npd, 128], BF16)
    d_T2 = nc.dram_tensor("T2", [ng, 128], BF16, addr_space="Shared")

    with tile.TileContext(nc) as tc, ExitStack() as top:
        cp = top.enter_context(tc.tile_pool(name="consts", bufs=1))
        pers = top.enter_context(tc.tile_pool(name="persist", bufs=1))

        W1sb = cp.tile([128, 72], BF16)
        W2sb = cp.tile([64, 72], BF16)
        VeBD = cp.tile([128, 64], BF16)
        LWsb = cp.tile([64, OUT], BF16)
        Isb = cp.tile([128, 128], F32)
        maskb = cp.tile([128, 16], BF16)
        b1bc = cp.tile([128, 64], F32)
        b2bc = cp.tile([128, 64], F32)
        lbbc = cp.tile([128, OUT], F32)
        iota = cp.tile([128, NSLOT], F32)
        iotab = cp.tile([128, NSLOT], BF16)
        Ib16 = cp.tile([128, 128], BF16)
        nc.sync.dma_start(W1sb[:], t_W1[:, :])
        nc.sync.dma_start(W2sb[:], t_W2[:, :])
        nc.sync.dma_start(VeBD[:], t_VeBD[:, :])
        nc.sync.dma_start(LWsb[:], t_LW[:, :])
        nc.sync.dma_start(Isb[:], t_I[:, :])
        nc.sync.dma_start(maskb[:], t_mask[:, :])
        nc.sync.dma_start(b1bc[:], t_cst[:, 0:64].partition_broadcast(128))
        nc.sync.dma_start(b2bc[:], t_cst[:, 64:128].partition_broadcast(128))
        nc.sync.dma_start(lbbc[:], t_cst[:, 128:128 + OUT].partition_broadcast(128))
        nc.sync.dma_start(iota[:], t_cst[:, 168:168 + NSLOT].partition_broadcast(128))
        nc.vector.tensor_copy(out=Ib16[:], in_=Isb[:])
        nc.vector.tensor_copy(out=iotab[:], in_=iota[:])

        # persistent SBUF state
        ae2sb = pers.tile([128, tq, 4], BF16)       # layer-2 a_e per edge
        ad1sb = pers.tile([128, jpd, 4], BF16)      # layer-1 a_d per local node
        ad2sb = pers.tile([128, jpd, 4], BF16)      # layer-2 a_d per local node
        zall = pers.tile([128, qpd, OUT], F32)      # head logits (shifted)
        smsb = pers.tile([128, qpd], F32)           # softmax sums

        # ---------------- phase A1: T1 = [x@W1 | a_s1]; local a_d1 ----------
        with ExitStack() as ph:
            ap = ph.enter_context(tc.tile_pool(name="pa_sb", bufs=3))
            app = ph.enter_context(tc.tile_pool(name="pa_ps", bufs=2, space="PSUM"))
            for it, i0 in enumerate(range(0, nt, 8)):
                bs = min(8, nt - i0)
                xt = ap.tile([128, 8 * 128], BF16, tag="xt")
                nc.sync.dma_start(xt[:, 0:128 * bs], t_xT[:, 128 * i0:128 * (i0 + bs)])
                ps0 = app.tile([128, 4, 72], F32, tag="ps0")
                ps1 = app.tile([128, 4, 72], F32, tag="ps1")
                for c in range(bs):
                    pst = ps0 if c < 4 else ps1
                    nc.tensor.matmul(pst[:, c % 4, :], xt[:, 128 * c:128 * (c + 1)],
                                     W1sb[:], start=True, stop=True)
                hsb = ap.tile([128, 8, 128], BF16, tag="hsb")
                nc.vector.tensor_copy(out=hsb[:, 0:4, 0:68], in_=ps0[:, :, 0:68])
                if bs > 4:
                    nc.vector.tensor_copy(out=hsb[:, 4:bs, 0:68],
                                          in_=ps1[:, 0:bs - 4, 0:68])
                eng = nc.gpsimd if it % 2 == 0 else nc.scalar
                eng.dma_start(
                    d_T1.ap()[128 * i0:128 * (i0 + bs), :].rearrange(
                        "(c r) d -> r c d", c=bs),
                    hsb[:, 0:bs, :])
            for jj0 in range(0, jpd, 8):
                bs = min(8, jpd - jj0)
                xt = ap.tile([128, 8 * 128], BF16, tag="xt")
                nc.sync.dma_start(xt[:, 0:128 * bs], t_xTl[:, 128 * jj0:128 * (jj0 + bs)])
                psa = app.tile([128, 32], F32, tag="psa")
                for c in range(bs):
                    nc.tensor.matmul(psa[:, 4 * c:4 * (c + 1)],
                                     xt[:, 128 * c:128 * (c + 1)],
                                     W1sb[:, 68:72], start=True, stop=True)
                nc.vector.tensor_copy(
                    out=ad1sb[:, jj0:jj0 + bs, :],
                    in_=psa[:, 0:4 * bs].rearrange("p (c v) -> p c v", v=4))

        # ---------------- edge phase (shared for both layers) ----------------
        def edge_layer(layer, tbl, adsb):
            with ExitStack() as ph:
                ip = ph.enter_context(tc.tile_pool(name=f"l{layer}_idx", bufs=3))
                gp = ph.enter_context(tc.tile_pool(name=f"l{layer}_g", bufs=3))
                sp = ph.enter_context(tc.tile_pool(name=f"l{layer}_s", bufs=2))
                mp = ph.enter_context(tc.tile_pool(name=f"l{layer}_m", bufs=2))
                ep = ph.enter_context(tc.tile_pool(name=f"l{layer}_e", bufs=2))
                pp = ph.enter_context(tc.tile_pool(name=f"l{layer}_ps", bufs=2, space="PSUM"))
                p1 = ph.enter_context(tc.tile_pool(name=f"l{layer}_p1", bufs=1, space="PSUM"))

                tlo = tbl.ap()[0:ng // 2, :]
                thi = tbl.ap()[ng // 2:ng, :]

                for q in range(qpd):
                    c0 = ch * q
                    comb = ip.tile([128, cw], I16, tag="comb")
                    nc.sync.dma_start(comb[:], t_comb[:, cw * q:cw * (q + 1)])
                    idxAv = comb[:, 0:cwA]
                    idxBv = comb[:, cwA:cwA + cwB]
                    m4 = comb[:, cwA + cwB:cwA + cwB + 4 * ch].bitcast(BF16)
                    drelb = comb[:, cwA + cwB + 4 * ch:cw].bitcast(BF16)
                    m4v = m4.rearrange("p (j b v) -> p j b v", j=4, v=4)

                    gA = gp.tile([128, uA, 128], BF16, tag="gA")
                    nc.gpsimd.dma_gather(
                        out_ap=gA[:], in_ap=tlo, idxs_ap=idxAv,
                        num_idxs=uA * 128, num_idxs_reg=uA * 128, elem_size=128,
                        single_packet=False)
                    gB = gp.tile([128, uB, 128], BF16, tag="gB")
                    nc.gpsimd.dma_gather(
                        out_ap=gB[:], in_ap=thi, idxs_ap=idxBv,
                        num_idxs=uB * 128, num_idxs_reg=uB * 128, elem_size=128,
                        single_packet=False)
                    gAv = gA[:].rearrange("p (j u) d -> p j u d", j=4)
                    gBv = gB[:].rearrange("p (j u) d -> p j u d", j=4)

                    # --- one-hot S, batch-major: [128, tps(b), 4(j), NSLOT]
                    S = sp.tile([128, tps, 4, NSLOT], BF16, tag="S")
                    nc.vector.tensor_tensor(
                        out=S[:],
                        in0=drelb.rearrange("p (j b) -> p b j", b=tps)
                            .unsqueeze(3).to_broadcast([128, tps, 4, NSLOT]),
                        in1=iotab[:].unsqueeze(1).unsqueeze(1)
                            .to_broadcast([128, tps, 4, NSLOT]),
                        op=ALU.is_equal)

                    # --- a_d expansion: S^T via PE, block-diag a_d matmul
                    bd = ip.tile([128, 16], BF16, tag="bd")
                    nc.vector.tensor_tensor(
                        out=bd[:],
                        in0=adsb[:, q, :].unsqueeze(1).to_broadcast([128, 4, 4]),
                        in1=maskb[:].rearrange("p (j v) -> p j v", v=4),
                        op=ALU.mult)
                    alad = p1.tile([128, tps * 16], F32, tag="alad")
                    for b0 in range(0, tps, 2):
                        stp = p1.tile([128, 256], BF16, tag="stp")
                        for b in (b0, b0 + 1):
                            nc.tensor.transpose(
                                stp[:, 128 * (b - b0):128 * (b - b0 + 1)],
                                S[:, b, :, :].rearrange("p a w -> p (a w)"), Ib16[:])
                        sts = sp.tile([128, 256], BF16, tag="sts")
                        nc.scalar.activation(sts[:], stp[:], ACT.Copy)
                        for b in (b0, b0 + 1):
                            nc.tensor.matmul(alad[:, 16 * b:16 * (b + 1)],
                                             sts[:, 128 * (b - b0):128 * (b - b0 + 1)],
                                             bd[:], start=True, stop=True)
                    aladb = ep.tile([128, tps * 16], BF16, tag="aladb")
                    nc.scalar.activation(aladb[:], alad[:], ACT.Copy)

                    # --- a_e
                    if layer == 1:
                        eac = ip.tile([128, 128 * ngr], BF16, tag="eac")
                        nc.sync.dma_start(eac[:], t_eaTg[:, 128 * ngr * q:128 * ngr * (q + 1)])
                        aeT = p1.tile([64, 128 * ngr], F32, tag="aeT")
                        for gi in range(ngr):
                            nc.tensor.matmul(aeT[:, 128 * gi:128 * (gi + 1)], VeBD[:],
                                             eac[:, 128 * gi:128 * (gi + 1)],
                                             start=True, stop=True)
                        aeTs = ep.tile([64, 128 * ngr], BF16, tag="aeTs")
                        nc.scalar.activation(aeTs[:], aeT[:], ACT.Copy)
                        aeps = p1.tile([128, 64 * ngr], F32, tag="aeps")
                        for gi in range(ngr):
                            nc.tensor.matmul(aeps[:, 64 * gi:64 * (gi + 1)],
                                             aeTs[:, 128 * gi:128 * (gi + 1)],
                                             Ib16[0:64, 0:64], start=True, stop=True)
                        aesb = ep.tile([128, ngr, 8, 8], BF16, tag="aesb")
                        nc.scalar.activation(
                            aesb[:].rearrange("p a b c -> p (a b c)"),
                            aeps[:], ACT.Copy)
                        nc.vector.tensor_copy(
                            out=ae2sb[:, c0:c0 + ch, :],
                            in_=aesb[:, :, :, 4:8])

                    # --- alpha = a_s[src](half-sel) + a_d[dst] + a_e
                    al = mp.tile([128, ch, 4], BF16, tag="al")
                    alv = al[:].rearrange("p (j b) v -> p j b v", j=4)
                    if nB0 > 0:
                        nc.vector.tensor_copy(out=alv[:, :, 0:nB0, :],
                                              in_=gAv[:, :, 0:nB0, 64:68])
                    if nA < tps:
                        nc.vector.tensor_copy(out=alv[:, :, nA:tps, :],
                                              in_=gBv[:, :, ov:nBn, 64:68])
                    if ov > 0:
                        tmp = mp.tile([128, 4, ov, 4], BF16, tag="tmp")
                        nc.vector.tensor_tensor(
                            out=tmp[:], in0=gBv[:, :, 0:ov, 64:68],
                            in1=gAv[:, :, nB0:nA, 64:68], op=ALU.subtract)
                        nc.vector.tensor_tensor(
                            out=tmp[:], in0=tmp[:],
                            in1=m4v[:, :, nB0:nA, :], op=ALU.mult)
                        nc.vector.tensor_tensor(
                            out=alv[:, :, nB0:nA, :], in0=gAv[:, :, nB0:nA, 64:68],
                            in1=tmp[:], op=ALU.add)
                    if layer == 1:
                        nc.vector.tensor_tensor(out=al[:], in0=al[:],
                                                in1=aesb[:, :, :, 0:4], op=ALU.add)
                    else:
                        nc.vector.tensor_tensor(out=al[:], in0=al[:],
                                                in1=ae2sb[:, c0:c0 + ch, :], op=ALU.add)
                    # += a_d (tile (j, b) lives at aladb[:, 16b + 4j : +4])
                    nc.vector.tensor_tensor(
                        out=al[:], in0=al[:],
                        in1=aladb[:].rearrange("p (b j v) -> p j b v", j=4, v=4),
                        op=ALU.add)
                    # leaky relu + exp (bf16)
                    lk = mp.tile([128, ch, 4], BF16, tag="lk")
                    nc.vector.tensor_scalar_mul(lk[:], al[:], NEG)
                    nc.vector.tensor_tensor(out=lk[:], in0=al[:], in1=lk[:], op=ALU.max)
                    exb = mp.tile([128, ch, 4], BF16, tag="exb")
                    nc.scalar.activation(exb[:], lk[:], ACT.Exp)
                    exbv = exb[:].rearrange("p (j b) v -> p j b v", j=4)

                    # region-masked exp weights: exLo (A tiles), exHi (B tiles)
                    exHi = mp.tile([128, 4, nBn, 4], BF16, tag="exHi")
                    if nA < tps:
                        nc.vector.tensor_copy(out=exHi[:, :, ov:nBn, :],
                                              in_=exbv[:, :, nA:tps, :])
                    if ov > 0:
                        nc.vector.tensor_tensor(
                            out=exHi[:, :, 0:ov, :], in0=exbv[:, :, nB0:nA, :],
                            in1=m4v[:, :, nB0:nA, :], op=ALU.mult)
                    exLo = mp.tile([128, 4, nA, 4], BF16, tag="exLo")
                    if nB0 > 0:
                        nc.vector.tensor_copy(out=exLo[:, :, 0:nB0, :],
                                              in_=exbv[:, :, 0:nB0, :])
                    if ov > 0:
                        nc.vector.tensor_tensor(
                            out=exLo[:, :, nB0:nA, :], in0=exbv[:, :, nB0:nA, :],
                            in1=exHi[:, :, 0:ov, :], op=ALU.subtract)
                    exLo2 = mp.tile([128, uA, 4, 2], BF16, tag="exLo2")
                    nc.scalar.activation(
                        exLo2[:],
                        exLo[:].rearrange("p j u v -> p (j u) v")
                        .unsqueeze(3).to_broadcast([128, uA, 4, 2]), ACT.Copy)
                    exHi2 = mp.tile([128, uB, 4, 2], BF16, tag="exHi2")
                    nc.scalar.activation(
                        exHi2[:],
                        exHi[:].rearrange("p j u v -> p (j u) v")
                        .unsqueeze(3).to_broadcast([128, uB, 4, 2]), ACT.Copy)

                    # --- messages: [h*ex (64) | ex (4)] per gather region
                    msgA = mp.tile([128, uA, 68], BF16, tag="msgA")
                    nc.vector.tensor_tensor(
                        out=msgA[:, :, 0:64].rearrange("p t (h c e) -> p t h c e",
                                                       h=H, e=2),
                        in0=gA[:, :, 0:64].rearrange("p t (h c e) -> p t h c e",
                                                     h=H, e=2),
                        in1=exLo2[:].unsqueeze(3).to_broadcast([128, uA, 4, 8, 2]),
                        op=ALU.mult)
                    nc.vector.tensor_copy(
                        out=msgA[:, :, 64:68],
                        in_=exLo[:].rearrange("p j u v -> p (j u) v"))
                    msgB = mp.tile([128, uB, 68], BF16, tag="msgB")
                    nc.vector.tensor_tensor(
                        out=msgB[:, :, 0:64].rearrange("p t (h c e) -> p t h c e",
                                                       h=H, e=2),
                        in0=gB[:, :, 0:64].rearrange("p t (h c e) -> p t h c e",
                                                     h=H, e=2),
                        in1=exHi2[:].unsqueeze(3).to_broadcast([128, uB, 4, 8, 2]),
                        op=ALU.mult)
                    nc.vector.tensor_copy(
                        out=msgB[:, :, 64:68],
                        in_=exHi[:].rearrange("p j u v -> p (j u) v"))

                    # --- aggregate per slot: 68-wide PSUM, 4 row blocks
                    U = pp.tile([128, 68], F32, tag="U")
                    for j in range(4):
                        for tt in range(nA):
                            nc.tensor.matmul(U[32 * j:32 * (j + 1), :],
                                             S[:, tt, j, :], msgA[:, j * nA + tt, :],
                                             start=(tt == 0), stop=False,
                                             tile_position=(0, 32 * j))
                        for tt in range(nB0, tps):
                            nc.tensor.matmul(U[32 * j:32 * (j + 1), :],
                                             S[:, tt, j, :],
                                             msgB[:, j * nBn + tt - nB0, :],
                                             start=False, stop=(tt == tps - 1),
                                             tile_position=(0, 32 * j))

                    # --- epilogue: out = U/(den+eps) + bias, relu
                    Usb = ep.tile([128, 64], F32, tag="Usb")
                    nc.vector.tensor_copy(out=Usb[:], in_=U[:, 0:64])
                    rec = ep.tile([128, 4], F32, tag="rec")
                    nc.vector.tensor_scalar_add(rec[:], U[:, 64:68], EPS)
                    nc.vector.reciprocal(rec[:], rec[:])
                    outq = ep.tile([128, 64], F32, tag="outq")
                    nc.vector.tensor_tensor(
                        out=outq[:].rearrange("p (h c) -> p h c", h=H),
                        in0=Usb[:].rearrange("p (h c) -> p h c", h=H),
                        in1=rec[:].unsqueeze(2).to_broadcast([128, H, HID]),
                        op=ALU.mult)
                    bias = b1bc if layer == 1 else b2bc
                    nc.vector.tensor_tensor(out=outq[:], in0=outq[:], in1=bias[:],
                                            op=ALU.add)
                    nc.vector.tensor_scalar_max(outq[:], outq[:], 0.0)

                    # transpose out_quad (both layers need it)
                    tp = p1.tile([128, 128], F32, tag="tp")
                    nc.tensor.transpose(tp[0:64, :], outq[:], Isb[:])
                    tpsb = ep.tile([64, 128], BF16, tag="tpsb")
                    nc.scalar.activation(tpsb[:], tp[0:64, :], ACT.Copy)

                    if layer == 1:
                        # layer-2 table rows + a_d2 for this quad's 128 nodes
                        t2ps = p1.tile([128, 72], F32, tag="t2ps")
                        nc.tensor.matmul(t2ps[:], tpsb[:], W2sb[:], start=True, stop=True)
                        t2sb = ep.tile([128, 128], BF16, tag="t2sb")
                        nc.scalar.activation(t2sb[:, 0:68], t2ps[:, 0:68], ACT.Copy)
                        nc.vector.tensor_copy(out=ad2sb[:, q, :], in_=t2ps[:, 68:72])
                        nc.sync.dma_start(
                            d_T2loc.ap()[128 * q:128 * (q + 1), :], t2sb[:])
                    else:
                        # head: logits into zall; exp-sum into smsb
                        lg = p1.tile([128, OUT], F32, tag="lg")
                        nc.tensor.matmul(lg[:], tpsb[:], LWsb[:], start=True, stop=True)
                        nc.vector.tensor_tensor(out=zall[:, q, :], in0=lg[:],
                                                in1=lbbc[:], op=ALU.add)
                        mx = ep.tile([128, 1], F32, tag="mx")
                        nc.vector.reduce_max(mx[:], zall[:, q, :],
                                             axis=mybir.AxisListType.X)
                        nc.vector.tensor_scalar(out=zall[:, q, :], in0=zall[:, q, :],
                                                scalar1=mx[:], scalar2=None,
                                                op0=ALU.subtract)
                        ez = ep.tile([128, OUT], BF16, tag="ez")
                        nc.scalar.activation(ez[:], zall[:, q, :], ACT.Exp,
                                             accum_out=smsb[:, q:q + 1])

        edge_layer(1, d_T1, ad1sb)

        # one AllGather of the layer-2 table (device-major concat)
        nc.gpsimd.collective_compute(
            "AllGather", mybir.AluOpType.bypass,
            replica_groups=[list(range(C))],
            ins=[d_T2loc.ap()],
            outs=[d_T2.ap()],
        )

        edge_layer(2, d_T2, ad2sb)

        # ---------------- tail: log_softmax finish + single output DMA -------
        with ExitStack() as ph:
            tpool = ph.enter_context(tc.tile_pool(name="tail", bufs=1))
            lnall = tpool.tile([128, qpd], F32)
            nc.scalar.activation(lnall[:], smsb[:], ACT.Ln)
            nc.vector.tensor_tensor(
                out=zall[:],
                in0=zall[:],
                in1=lnall[:].unsqueeze(2).to_broadcast([128, qpd, OUT]),
                op=ALU.subtract)
            nc.sync.dma_start(t_out[:, :],
                              zall[:].rearrange("p q d -> p (q d)"))

    return nc


# ----------------------------------------------------------------------------
# public entry
# ----------------------------------------------------------------------------

def _prepare(inputs):
    x = np.asarray(inputs["x"], np.float32)
    ei = np.asarray(inputs["edge_index"], np.int64)
    ea = np.asarray(inputs["edge_attr"], np.float32)
    n = x.shape[0]
    loop = np.arange(n, dtype=np.int64)
    src = np.concatenate([ei[0], loop])
    dst = np.concatenate([ei[1], loop])
    mean_attr = ea.mean(axis=0)

    plan = _build_plan(src, dst, n)
    per_core, xT, tq = _host_arrays(plan, x, src, dst, ea, mean_attr, n)

    W1aug, W2aug, VeBD, LW, lb2p, b1, b2 = _fold_weights(
        np.asarray(inputs["W1"], np.float32), np.asarray(inputs["att_src1"], np.float32),
        np.asarray(inputs["att_dst1"], np.float32), np.asarray(inputs["We1"], np.float32),
        np.asarray(inputs["att_e1"], np.float32), np.asarray(inputs["b1"], np.float32),
        np.asarray(inputs["W2"], np.float32), np.asarray(inputs["att_src2"], np.float32),
        np.asarray(inputs["att_dst2"], np.float32), np.asarray(inputs["We2"], np.float32),
        np.asarray(inputs["att_e2"], np.float32), np.asarray(inputs["b2"], np.float32),
        np.asarray(inputs["lw1"], np.float32), np.asarray(inputs["lb1"], np.float32),
        np.asarray(inputs["lw2"], np.float32), np.asarray(inputs["lb2"], np.float32))

    bf16 = _bf16()
    cst = np.zeros((1, 256), np.float32)
    cst[0, 0:64] = b1
    cst[0, 64:128] = b2
    cst[0, 128:128 + OUT] = lb2p
    cst[0, 168:168 + NSLOT] = np.arange(NSLOT, dtype=np.float32)
    ident = np.eye(128, dtype=np.float32)
    bdmask = np.zeros((128, 16), np.float32)
    for j in range(4):
        bdmask[32 * j:32 * (j + 1), 4 * j:4 * (j + 1)] = 1.0
    bdmask = bdmask.astype(bf16)

    in_maps = []
    for d in range(C):
        pc = per_core[d]
        in_maps.append({
            "xT": xT, "xTloc": pc["xTloc"], "eaTg": pc["eaTg"], "comb": pc["comb"],
            "W1aug": W1aug, "W2aug": W2aug, "VeBD": VeBD, "LW": LW, "cst": cst,
            "ident": ident, "bdmask": bdmask,
        })
    return plan, in_maps, tq


def _assemble(plan, outs, n):
    bin_of, pos_of, spd, qpd = plan["bin_of"], plan["pos_of"], plan["spd"], plan["qpd"]
    dev = bin_of // spd
    s = bin_of % spd
    q = s // 4
    u = (s % 4) * NSLOT + pos_of
    stacked = np.stack([np.asarray(o, np.float32).reshape(128, qpd, OUT)
                        for o in outs])
    return stacked[dev[:n], u[:n], q[:n]]


def _run(inputs, trace=False, **spmd_kwargs):
    from concourse.bass_utils import run_bass_kernel_spmd

    plan, in_maps, tq = _prepare(inputs)
    nc = _build_nc(plan["ng"], plan["npd"], plan["spd"], plan["tps"], tq,
                   plan["nA"], plan["nB0"])
    nc.compile()
    res = run_bass_kernel_spmd(nc, in_maps, core_ids=list(range(C)), trace=trace,
                               **spmd_kwargs)
    outs = [r["out"] for r in res.results]
    return _assemble(plan, outs, inputs["x"].shape[0]), res


def kernel(**inputs):
    out, _ = _run(inputs)
    return out


# revision 24
# speedup vs baseline: 2.7305x; 1.3288x over previous
"""GATNet (2x GATConv + MLP head + log_softmax) on 8 Trainium2 NeuronCores.

Strategy (dst-partitioned message passing, v3):
  - Host assigns destination nodes to 8 devices x SPD slots (32 nodes/slot),
    balancing in-edge counts so every slot has exactly TPS 128-edge tiles.
    Every device runs an identical program; per-device data differ.
  - Layer tables T = [h(64) | a_s(4) | pad] are single 256B bf16 rows, one per
    node, [ng, 128].  int16 gather indices only reach 32767, so each slot's
    edges are sorted low-half sources first and every chunk runs TWO
    dma_gathers: A over tiles [0, nA) against table rows [0, ng/2), B over
    tiles [nB0, tps) against rows [ng/2, ng).  Only ~(nA+tps-nB0)/tps of the
    pair-gather bytes move, which is what the gather costs on the Pool queue.
  - Per chunk (4 slots = 128 dst nodes) attention is built edge-wise (bf16),
    low/high exp weights are region-masked, and messages aggregate into one
    68-wide PSUM accumulator at 4 partition offsets via one-hot matmuls.
  - Layer-2 table rows are produced inside layer 1's epilogue (each device
    computes rows only for its own nodes = device-major global numbering) and
    shared with a single AllGather between the layers.
  - a_e for both layers is computed once in layer 1 and parked in SBUF; a_d
    tables also stay resident in SBUF.  log_softmax's Ln runs once at the end
    (avoids per-chunk activation-table reloads); output leaves in one DMA.
"""

import numpy as np

# model constants (fixed by the problem)
IN = 128
HID = 16
OUT = 40
H = 4
ED = 16
HC = 64  # HID * H
NEG = 0.2
EPS = 1e-16

C = 8          # NeuronCores
NSLOT = 32     # nodes per slot (= one-hot width, PSUM col-block)


def _bf16():
    import concourse.mybir as mybir
    return mybir.dt.np(mybir.dt.bfloat16)


# ----------------------------------------------------------------------------
# host-side plan: balance nodes into (device, slot) bins, lay out edge shards
# ----------------------------------------------------------------------------

def _build_plan(src, dst, n_nodes):
    """Returns a dict with the full sharding plan. src/dst include self-loops."""
    import heapq

    deg = np.bincount(dst, minlength=n_nodes).astype(np.int64)
    e_tot = src.shape[0]

    def try_pack(nbins, cap_e, lo_deg=None, cap_lo=None, cap_hi=None):
        # LPT: heaviest nodes first into least-loaded feasible bin; optional
        # second dimension caps the low-half / high-half in-edge loads.
        order = np.argsort(-deg, kind="stable")
        loads = [(0, b) for b in range(nbins)]
        heapq.heapify(loads)
        bin_of_t = np.empty(n_nodes, np.int64)
        bin_cnt = np.zeros(nbins, np.int64)
        bin_load = np.zeros(nbins, np.int64)
        bin_lo = np.zeros(nbins, np.int64)
        for nd in order:
            d = int(deg[nd])
            lo = int(lo_deg[nd]) if lo_deg is not None else 0
            hi = d - lo
            spill = []
            placed = False
            while loads:
                l, b = heapq.heappop(loads)
                ok = bin_cnt[b] < NSLOT and bin_load[b] + d <= cap_e
                if ok and lo_deg is not None:
                    ok = (bin_lo[b] + lo <= cap_lo
                          and (bin_load[b] + d) - (bin_lo[b] + lo) <= cap_hi)
                if ok:
                    bin_of_t[nd] = b
                    bin_cnt[b] += 1
                    bin_load[b] += d
                    bin_lo[b] += lo
                    heapq.heappush(loads, (bin_load[b], b))
                    placed = True
                    break
                elif bin_cnt[b] < NSLOT:
                    spill.append((l, b))
                # full bins are dropped
            for it in spill:
                heapq.heappush(loads, it)
            if not placed:
                return None
        return bin_of_t

    # search (slots-per-device, even tiles-per-slot) minimizing total tiles
    spd_min = 4 * int(np.ceil(n_nodes / (C * NSLOT * 4)))  # node-capacity floor
    best = None  # (tq, spd, tps, bin_of)
    for spd_try in range(spd_min, spd_min + 65, 4):
        nbins = C * spd_try
        tps_lo = int(np.ceil(e_tot / nbins / 128.0))
        tps_lo += tps_lo % 2  # ch = 4*tps must be a multiple of 8
        for tps_try in (tps_lo, tps_lo + 2):
            if best is not None and spd_try * tps_try >= best[0]:
                continue
            got = try_pack(nbins, tps_try * 128)
            if got is not None:
                best = (spd_try * tps_try, spd_try, tps_try, got)
                break
        if best is not None and (spd_try + 4) * 2 >= best[0]:
            break
    assert best is not None, "balancer failed"
    _, spd, tps, bin_of = best

    nbins = C * spd
    npd = spd * NSLOT
    ng = C * npd

    def finish(bin_of):
        # position of each node within its bin; device-major global numbering
        pos_of = np.zeros(n_nodes, np.int64)
        fill = np.zeros(nbins, np.int64)
        for nd in range(n_nodes):
            b = bin_of[nd]
            pos_of[nd] = fill[b]
            fill[b] += 1
        node2g = ((bin_of // spd) * npd + (bin_of % spd) * NSLOT + pos_of
                  ).astype(np.int64)
        return pos_of, node2g

    pos_of, node2g = finish(bin_of)

    # table half boundary: low rows [0, X), high rows [X, ng).  Both gather
    # index spaces must fit int16.
    X = int(round(0.4435 * ng / 128.0)) * 128
    X = min(X, 32768)
    assert ng - X <= 32768

    # re-pack with per-bin low/high in-edge caps so the two gather spans are
    # as narrow as possible (lo <= 500 -> nA=4; hi <= 615 -> nB0>=3 for full
    # bins).  Low-degrees come from the first pack's numbering; the re-pack
    # perturbs halves only slightly, so re-measure and take actual spans.
    lo_src = (node2g[src] < X)
    lo_deg_nd = np.bincount(dst[lo_src], minlength=n_nodes).astype(np.int64)
    repack = try_pack(nbins, tps * 128, lo_deg=lo_deg_nd, cap_lo=500, cap_hi=615)
    if repack is not None:
        bin_of = repack
        pos_of, node2g = finish(bin_of)

    # edges per destination bin, LOW-half sources first, then high, then pad
    ebin = bin_of[dst]
    half = (node2g[src] >= X).astype(np.int64)
    order = np.lexsort((half, ebin))          # by bin, low-src first (stable)
    counts = np.bincount(ebin, minlength=nbins)
    lowcnt = np.bincount(ebin[half == 0], minlength=nbins)
    cap = tps * 128
    assert counts.max() <= cap
    starts = np.zeros(nbins + 1, np.int64)
    np.cumsum(counts, out=starts[1:])
    rank = np.arange(e_tot, dtype=np.int64) - starts[ebin[order]]
    canvas = np.full((nbins, cap), -1, np.int64)       # edge id or -1 pad
    canvas[ebin[order], rank] = order

    # static gather spans: A covers tiles [0, nA), B covers [nB0, tps)
    nA = int(np.ceil(lowcnt.max() / 128.0))
    nB0 = int(lowcnt.min() // 128)
    assert 0 <= nB0 <= nA <= tps

    return dict(
        spd=spd, tps=tps, npd=npd, ng=ng, nbins=nbins, qpd=spd // 4,
        nA=nA, nB0=nB0, X=X, bin_of=bin_of, pos_of=pos_of, node2g=node2g,
        canvas=canvas,
    )


def _host_arrays(plan, x, src, dst, edge_attr, mean_attr, n_nodes):
    """Per-core input arrays."""
    bf16 = _bf16()
    spd, tps, npd, ng = plan["spd"], plan["tps"], plan["npd"], plan["ng"]
    nA, nB0, X = plan["nA"], plan["nB0"], plan["X"]
    nBn = tps - nB0
    node2g, pos_of, canvas = plan["node2g"], plan["pos_of"], plan["canvas"]
    tq = spd * tps                       # 128-edge tiles per device
    ch = 4 * tps
    nq = tq // ch                        # chunks
    e0 = edge_attr.shape[0]

    def wrap16(a):  # [n] i16 -> [128, n//16] gather-index wrapping
        return np.tile(a.reshape(-1, 16).T, (8, 1))

    # permuted node features, transposed: xT [IN, ng] (bf16)
    xg = np.zeros((ng, IN), np.float32)
    xg[node2g] = np.asarray(x, np.float32)
    xT = np.ascontiguousarray(xg.T.astype(bf16))

    per_core = []
    for d in range(C):
        cv = canvas[d * spd:(d + 1) * spd].reshape(tq, 128)  # [tile, lane]
        valid = cv >= 0
        eid = np.where(valid, cv, 0)
        srcg = np.where(valid, node2g[src[eid]], 0)          # [tq, 128]
        hi = (srcg >= X)
        idxA = np.where(hi, 0, srcg).astype(np.int16)
        idxB = np.where(hi, srcg - X, 0).astype(np.int16)
        m4 = np.repeat(hi.astype(bf16).T[:, :, None], 4, axis=2
                       ).reshape(128, tq * 4).view(np.int16)  # [128, tq*4]
        drel = np.where(valid, pos_of[dst[eid]], -1)          # [tq, 128]
        # host-built one-hot S: [p, chunk, (b j s)] (bf16 bits)
        S_all = (drel[:, :, None] == np.arange(NSLOT)).astype(bf16)
        Sfull = (S_all.reshape(nq, 4, tps, 128, NSLOT)
                 .transpose(3, 0, 2, 1, 4)
                 .reshape(128, nq, tps * 4 * NSLOT)).view(np.int16)

        # per-chunk comb block: [idxA | idxB | m4 | S] int16
        cwA, cwB = 32 * nA, 32 * nBn
        cw = cwA + cwB + 4 * ch + NSLOT * ch
        comb = np.empty((128, nq * cw), np.int16)
        t4 = np.arange(tq).reshape(nq, 4, tps)                # chunk, j, tt
        for q in range(nq):
            blk = comb[:, q * cw:(q + 1) * cw]
            tA = t4[q, :, 0:nA].reshape(-1)                   # u = j*nA+tt
            tB = t4[q, :, nB0:tps].reshape(-1)
            blk[:, 0:cwA] = wrap16(idxA[tA].reshape(-1))
            blk[:, cwA:cwA + cwB] = wrap16(idxB[tB].reshape(-1))
            blk[:, cwA + cwB:cwA + cwB + 4 * ch] = m4[:, q * 4 * ch:(q + 1) * 4 * ch]
            blk[:, cwA + cwB + 4 * ch:] = Sfull[:, q, :]

        ea = np.zeros((tq, 128, ED), np.float32)
        sel = valid & (eid < e0)
        ea[sel] = edge_attr[eid[sel]]
        loop_sel = valid & (eid >= e0)
        ea[loop_sel] = mean_attr
        # eaTg[(tt)*16 + r, g*128 + p]: groups of 8 tiles (bf16)
        eaTg = np.ascontiguousarray(
            ea.reshape(tq // 8, 8, 128, ED).transpose(1, 3, 0, 2)
            .reshape(128, (tq // 8) * 128).astype(bf16))
        per_core.append(dict(
            comb=np.ascontiguousarray(comb), eaTg=eaTg,
            xTloc=np.ascontiguousarray(xT[:, d * npd:(d + 1) * npd]),
        ))
    return per_core, xT, tq


def _fold_weights(W1, att_s1, att_d1, We1, att_e1, b1,
                  W2, att_s2, att_d2, We2, att_e2, b2,
                  lw1, lb1, lw2, lb2):
    bf16 = _bf16()

    def head_fold(att):  # [H, HID] -> [HC, H] block diag columns
        A = np.zeros((HC, H), np.float32)
        for h in range(H):
            A[h * HID:(h + 1) * HID, h] = att[h]
        return A

    W1aug = np.concatenate([W1, W1 @ head_fold(att_s1), W1 @ head_fold(att_d1)], 1)
    W2aug = np.concatenate([W2, W2 @ head_fold(att_s2), W2 @ head_fold(att_d2)], 1)
    Ve = np.zeros((ED, 8), np.float32)
    for h in range(H):
        Ve[:, h] = We1[:, h * HID:(h + 1) * HID] @ att_e1[h]
        Ve[:, 4 + h] = We2[:, h * HID:(h + 1) * HID] @ att_e2[h]
    VeBD = np.zeros((128, 64), np.float32)
    for j in range(8):
        VeBD[ED * j:ED * (j + 1), 8 * j:8 * (j + 1)] = Ve
    LW = (lw1 @ lw2).astype(np.float32)
    lb2p = (lb1 @ lw2 + lb2).astype(np.float32)
    return (W1aug.astype(bf16), W2aug.astype(bf16), VeBD.astype(bf16),
            LW.astype(bf16), lb2p, b1.astype(np.float32), b2.astype(np.float32))


# ----------------------------------------------------------------------------
# the bass program (identical for all cores)
# ----------------------------------------------------------------------------

def _build_nc(ng, npd, spd, tps, tq, nA, nB0, X):
    import concourse.bass as bass
    import concourse.mybir as mybir
    import concourse.tile as tile
    from concourse import bacc
    from contextlib import ExitStack

    F32 = mybir.dt.float32
    BF16 = mybir.dt.bfloat16
    I16 = mybir.dt.int16
    ALU = mybir.AluOpType
    ACT = mybir.ActivationFunctionType

    ch = 4 * tps          # tiles per chunk (one quad = 4 slots)
    qpd = spd // 4        # chunks per device per layer
    nt = ng // 128        # node tiles (table build)
    jpd = npd // 128      # local 128-node groups (== qpd)
    ngr = ch // 8         # eaTg groups per chunk
    nBn = tps - nB0
    ov = nA - nB0         # mixed tiles per slot
    uA, uB = 4 * nA, 4 * nBn
    cwA, cwB = 32 * nA, 32 * nBn
    cw = cwA + cwB + 4 * ch + NSLOT * ch

    nc = bacc.Bacc(None, target_bir_lowering=False)

    # kernel IO
    t_xT = nc.dram_tensor("xT", [128, ng], BF16, kind="ExternalInput")
    t_xTl = nc.dram_tensor("xTloc", [128, npd], BF16, kind="ExternalInput")
    t_eaTg = nc.dram_tensor("eaTg", [128, (tq // 8) * 128], BF16, kind="ExternalInput")
    t_comb = nc.dram_tensor("comb", [128, (tq // ch) * cw], I16, kind="ExternalInput")
    t_W1 = nc.dram_tensor("W1aug", [128, 72], BF16, kind="ExternalInput")
    t_W2 = nc.dram_tensor("W2aug", [64, 72], BF16, kind="ExternalInput")
    t_VeBD = nc.dram_tensor("VeBD", [128, 64], BF16, kind="ExternalInput")
    t_LW = nc.dram_tensor("LW", [64, OUT], BF16, kind="ExternalInput")
    t_cst = nc.dram_tensor("cst", [1, 256], F32, kind="ExternalInput")
    # cst row: [b1(64) | b2(64) | lb2p(40) | iota32(32) | pad]
    t_mask = nc.dram_tensor("bdmask", [128, 16], BF16, kind="ExternalInput")
    t_I = nc.dram_tensor("ident", [128, 128], F32, kind="ExternalInput")
    t_out = nc.dram_tensor("out", [128, qpd * OUT], F32, kind="ExternalOutput")

    # node tables: tight 136B rows for builds/collective; padded 256B rows
    # (DRAM->DRAM expanded) for the 256B-granularity gathers
    d_T1t = nc.dram_tensor("T1t", [ng, 68], BF16)
    d_T1 = nc.dram_tensor("T1", [ng, 128], BF16)
    d_T2loc = nc.dram_tensor("T2loc", [npd, 68], BF16)
    d_T2t = nc.dram_tensor("T2t", [ng, 68], BF16, addr_space="Shared")
    d_T2 = nc.dram_tensor("T2", [ng, 128], BF16)

    with tile.TileContext(nc) as tc, ExitStack() as top:
        cp = top.enter_context(tc.tile_pool(name="consts", bufs=1))
        pers = top.enter_context(tc.tile_pool(name="persist", bufs=1))

        W1sb = cp.tile([128, 72], BF16)
        W2sb = cp.tile([64, 72], BF16)
        VeBD = cp.tile([128, 64], BF16)
        LWsb = cp.tile([64, OUT], BF16)
        Isb = cp.tile([128, 128], F32)
        maskb = cp.tile([128, 16], BF16)
        b1bc = cp.tile([128, 64], F32)
        b2bc = cp.tile([128, 64], F32)
        lbbc = cp.tile([128, OUT], F32)
        Ib16 = cp.tile([128, 128], BF16)
        nc.sync.dma_start(W1sb[:], t_W1[:, :])
        nc.sync.dma_start(W2sb[:], t_W2[:, :])
        nc.sync.dma_start(VeBD[:], t_VeBD[:, :])
        nc.sync.dma_start(LWsb[:], t_LW[:, :])
        nc.sync.dma_start(Isb[:], t_I[:, :])
        nc.sync.dma_start(maskb[:], t_mask[:, :])
        nc.sync.dma_start(b1bc[:], t_cst[:, 0:64].partition_broadcast(128))
        nc.sync.dma_start(b2bc[:], t_cst[:, 64:128].partition_broadcast(128))
        nc.sync.dma_start(lbbc[:], t_cst[:, 128:128 + OUT].partition_broadcast(128))
        nc.vector.tensor_copy(out=Ib16[:], in_=Isb[:])

        # persistent SBUF state
        ae2sb = pers.tile([128, tq, 4], BF16)       # layer-2 a_e per edge
        ad1sb = pers.tile([128, jpd, 4], BF16)      # layer-1 a_d per local node
        ad2sb = pers.tile([128, jpd, 4], BF16)      # layer-2 a_d per local node
        zall = pers.tile([128, qpd, OUT], F32)      # head logits (shifted)
        smsb = pers.tile([128, qpd], F32)           # softmax sums

        # ---------------- phase A1: T1 = [x@W1 | a_s1]; local a_d1 ----------
        with ExitStack() as ph:
            ap = ph.enter_context(tc.tile_pool(name="pa_sb", bufs=3))
            app = ph.enter_context(tc.tile_pool(name="pa_ps", bufs=2, space="PSUM"))
            for it, i0 in enumerate(range(0, nt, 8)):
                bs = min(8, nt - i0)
                xt = ap.tile([128, 8 * 128], BF16, tag="xt")
                nc.sync.dma_start(xt[:, 0:128 * bs], t_xT[:, 128 * i0:128 * (i0 + bs)])
                ps0 = app.tile([128, 4, 72], F32, tag="ps0")
                ps1 = app.tile([128, 4, 72], F32, tag="ps1")
                for c in range(bs):
                    pst = ps0 if c < 4 else ps1
                    nc.tensor.matmul(pst[:, c % 4, :], xt[:, 128 * c:128 * (c + 1)],
                                     W1sb[:], start=True, stop=True)
                hsb = ap.tile([128, 8, 68], BF16, tag="hsb")
                nc.vector.tensor_copy(out=hsb[:, 0:4, :], in_=ps0[:, :, 0:68])
                if bs > 4:
                    nc.vector.tensor_copy(out=hsb[:, 4:bs, :],
                                          in_=ps1[:, 0:bs - 4, 0:68])
                eng = nc.gpsimd if it % 2 == 0 else nc.scalar
                eng.dma_start(
                    d_T1t.ap()[128 * i0:128 * (i0 + bs), :].rearrange(
                        "(c r) d -> r c d", c=bs),
                    hsb[:, 0:bs, :])
            for jj0 in range(0, jpd, 8):
                bs = min(8, jpd - jj0)
                xt = ap.tile([128, 8 * 128], BF16, tag="xt")
                nc.sync.dma_start(xt[:, 0:128 * bs], t_xTl[:, 128 * jj0:128 * (jj0 + bs)])
                psa = app.tile([128, 32], F32, tag="psa")
                for c in range(bs):
                    nc.tensor.matmul(psa[:, 4 * c:4 * (c + 1)],
                                     xt[:, 128 * c:128 * (c + 1)],
                                     W1sb[:, 68:72], start=True, stop=True)
                nc.vector.tensor_copy(
                    out=ad1sb[:, jj0:jj0 + bs, :],
                    in_=psa[:, 0:4 * bs].rearrange("p (c v) -> p c v", v=4))
            # expand tight rows into the 256B-granularity gather table
            nc.sync.dma_start(d_T1.ap()[:, 0:68], d_T1t.ap()[:, :])

        # ---------------- edge phase (shared for both layers) ----------------
        def edge_layer(layer, tbl, adsb):
            with ExitStack() as ph:
                ip = ph.enter_context(tc.tile_pool(name=f"l{layer}_idx", bufs=3))
                gp = ph.enter_context(tc.tile_pool(name=f"l{layer}_g", bufs=3))
                sp = ph.enter_context(tc.tile_pool(name=f"l{layer}_s", bufs=2))
                mp = ph.enter_context(tc.tile_pool(name=f"l{layer}_m", bufs=2))
                ep = ph.enter_context(tc.tile_pool(name=f"l{layer}_e", bufs=2))
                pp = ph.enter_context(tc.tile_pool(name=f"l{layer}_ps", bufs=2, space="PSUM"))
                p1 = ph.enter_context(tc.tile_pool(name=f"l{layer}_p1", bufs=1, space="PSUM"))

                tlo = tbl.ap()[0:X, :]
                thi = tbl.ap()[X:ng, :]

                for q in range(qpd):
                    c0 = ch * q
                    comb = ip.tile([128, cw], I16, tag="comb")
                    nc.sync.dma_start(comb[:], t_comb[:, cw * q:cw * (q + 1)])
                    idxAv = comb[:, 0:cwA]
                    idxBv = comb[:, cwA:cwA + cwB]
                    m4 = comb[:, cwA + cwB:cwA + cwB + 4 * ch].bitcast(BF16)
                    m4v = m4.rearrange("p (j b v) -> p j b v", j=4, v=4)
                    Sv = (comb[:, cwA + cwB + 4 * ch:cw].bitcast(BF16)
                          .rearrange("p (b j s) -> p b j s", j=4, s=NSLOT))

                    gA = gp.tile([128, uA, 128], BF16, tag="gA")
                    nc.gpsimd.dma_gather(
                        out_ap=gA[:], in_ap=tlo, idxs_ap=idxAv,
                        num_idxs=uA * 128, num_idxs_reg=uA * 128, elem_size=128,
                        single_packet=False)
                    gB = gp.tile([128, uB, 128], BF16, tag="gB")
                    nc.gpsimd.dma_gather(
                        out_ap=gB[:], in_ap=thi, idxs_ap=idxBv,
                        num_idxs=uB * 128, num_idxs_reg=uB * 128, elem_size=128,
                        single_packet=False)
                    gAv = gA[:].rearrange("p (j u) d -> p j u d", j=4)
                    gBv = gB[:].rearrange("p (j u) d -> p j u d", j=4)

                    # --- a_d expansion: S^T via PE, block-diag a_d matmul
                    bd = ip.tile([128, 16], BF16, tag="bd")
                    nc.vector.tensor_tensor(
                        out=bd[:],
                        in0=adsb[:, q, :].unsqueeze(1).to_broadcast([128, 4, 4]),
                        in1=maskb[:].rearrange("p (j v) -> p j v", v=4),
                        op=ALU.mult)
                    stp = p1.tile([128, tps * 128], BF16, tag="stp")
                    for b in range(tps):
                        nc.tensor.transpose(
                            stp[:, 128 * b:128 * (b + 1)],
                            Sv[:, b, :, :].rearrange("p a w -> p (a w)"), Ib16[:])
                    sts = sp.tile([128, tps * 128], BF16, tag="sts")
                    nc.scalar.activation(sts[:], stp[:], ACT.Copy)
                    alad = p1.tile([128, tps * 16], F32, tag="alad")
                    for b in range(tps):
                        nc.tensor.matmul(alad[:, 16 * b:16 * (b + 1)],
                                         sts[:, 128 * b:128 * (b + 1)],
                                         bd[:], start=True, stop=True)
                    aladb = ep.tile([128, tps * 16], BF16, tag="aladb")
                    nc.scalar.activation(aladb[:], alad[:], ACT.Copy)

                    # --- a_e
                    if layer == 1:
                        eac = ip.tile([128, 128 * ngr], BF16, tag="eac")
                        nc.sync.dma_start(eac[:], t_eaTg[:, 128 * ngr * q:128 * ngr * (q + 1)])
                        aeT = p1.tile([64, 128 * ngr], F32, tag="aeT")
                        for gi in range(ngr):
                            nc.tensor.matmul(aeT[:, 128 * gi:128 * (gi + 1)], VeBD[:],
                                             eac[:, 128 * gi:128 * (gi + 1)],
                                             start=True, stop=True)
                        aeTs = ep.tile([64, 128 * ngr], BF16, tag="aeTs")
                        nc.scalar.activation(aeTs[:], aeT[:], ACT.Copy)
                        aeps = p1.tile([128, 64 * ngr], F32, tag="aeps")
                        for gi in range(ngr):
                            nc.tensor.matmul(aeps[:, 64 * gi:64 * (gi + 1)],
                                             aeTs[:, 128 * gi:128 * (gi + 1)],
                                             Ib16[0:64, 0:64], start=True, stop=True)
                        aesb = ep.tile([128, ngr, 8, 8], BF16, tag="aesb")
                        nc.scalar.activation(
                            aesb[:].rearrange("p a b c -> p (a b c)"),
                            aeps[:], ACT.Copy)
                        nc.vector.tensor_copy(
                            out=ae2sb[:, c0:c0 + ch, :],
                            in_=aesb[:, :, :, 4:8])

                    # --- alpha = a_s[src](half-sel) + a_d[dst] + a_e
                    al = mp.tile([128, ch, 4], BF16, tag="al")
                    alv = al[:].rearrange("p (j b) v -> p j b v", j=4)
                    if nB0 > 0:
                        nc.vector.tensor_copy(out=alv[:, :, 0:nB0, :],
                                              in_=gAv[:, :, 0:nB0, 64:68])
                    if nA < tps:
                        nc.vector.tensor_copy(out=alv[:, :, nA:tps, :],
                                              in_=gBv[:, :, ov:nBn, 64:68])
                    if ov > 0:
                        tmp = mp.tile([128, 4, ov, 4], BF16, tag="tmp")
                        nc.vector.tensor_tensor(
                            out=tmp[:], in0=gBv[:, :, 0:ov, 64:68],
                            in1=gAv[:, :, nB0:nA, 64:68], op=ALU.subtract)
                        nc.vector.tensor_tensor(
                            out=tmp[:], in0=tmp[:],
                            in1=m4v[:, :, nB0:nA, :], op=ALU.mult)
                        nc.vector.tensor_tensor(
                            out=alv[:, :, nB0:nA, :], in0=gAv[:, :, nB0:nA, 64:68],
                            in1=tmp[:], op=ALU.add)
                    if layer == 1:
                        nc.vector.tensor_tensor(out=al[:], in0=al[:],
                                                in1=aesb[:, :, :, 0:4], op=ALU.add)
                    else:
                        nc.vector.tensor_tensor(out=al[:], in0=al[:],
                                                in1=ae2sb[:, c0:c0 + ch, :], op=ALU.add)
                    # += a_d (tile (j, b) lives at aladb[:, 16b + 4j : +4])
                    nc.vector.tensor_tensor(
                        out=al[:], in0=al[:],
                        in1=aladb[:].rearrange("p (b j v) -> p j b v", j=4, v=4),
                        op=ALU.add)
                    # leaky relu + exp (bf16)
                    lk = mp.tile([128, ch, 4], BF16, tag="lk")
                    nc.vector.tensor_scalar_mul(lk[:], al[:], NEG)
                    nc.vector.tensor_tensor(out=lk[:], in0=al[:], in1=lk[:], op=ALU.max)
                    exb = mp.tile([128, ch, 4], BF16, tag="exb")
                    nc.scalar.activation(exb[:], lk[:], ACT.Exp)
                    exbv = exb[:].rearrange("p (j b) v -> p j b v", j=4)

                    # region-masked exp weights: exLo (A tiles), exHi (B tiles)
                    exHi = mp.tile([128, 4, nBn, 4], BF16, tag="exHi")
                    if nA < tps:
                        nc.vector.tensor_copy(out=exHi[:, :, ov:nBn, :],
                                              in_=exbv[:, :, nA:tps, :])
                    if ov > 0:
                        nc.vector.tensor_tensor(
                            out=exHi[:, :, 0:ov, :], in0=exbv[:, :, nB0:nA, :],
                            in1=m4v[:, :, nB0:nA, :], op=ALU.mult)
                    exLo = mp.tile([128, 4, nA, 4], BF16, tag="exLo")
                    if nB0 > 0:
                        nc.vector.tensor_copy(out=exLo[:, :, 0:nB0, :],
                                              in_=exbv[:, :, 0:nB0, :])
                    if ov > 0:
                        nc.vector.tensor_tensor(
                            out=exLo[:, :, nB0:nA, :], in0=exbv[:, :, nB0:nA, :],
                            in1=exHi[:, :, 0:ov, :], op=ALU.subtract)
                    exLo2 = mp.tile([128, uA, 4, 2], BF16, tag="exLo2")
                    nc.scalar.activation(
                        exLo2[:],
                        exLo[:].rearrange("p j u v -> p (j u) v")
                        .unsqueeze(3).to_broadcast([128, uA, 4, 2]), ACT.Copy)
                    exHi2 = mp.tile([128, uB, 4, 2], BF16, tag="exHi2")
                    nc.scalar.activation(
                        exHi2[:],
                        exHi[:].rearrange("p j u v -> p (j u) v")
                        .unsqueeze(3).to_broadcast([128, uB, 4, 2]), ACT.Copy)

                    # --- messages: [h*ex (64) | ex (4)] per gather region
                    msgA = mp.tile([128, uA, 68], BF16, tag="msgA")
                    nc.vector.tensor_tensor(
                        out=msgA[:, :, 0:64].rearrange("p t (h c e) -> p t h c e",
                                                       h=H, e=2),
                        in0=gA[:, :, 0:64].rearrange("p t (h c e) -> p t h c e",
                                                     h=H, e=2),
                        in1=exLo2[:].unsqueeze(3).to_broadcast([128, uA, 4, 8, 2]),
                        op=ALU.mult)
                    nc.vector.tensor_copy(
                        out=msgA[:, :, 64:68],
                        in_=exLo[:].rearrange("p j u v -> p (j u) v"))
                    msgB = mp.tile([128, uB, 68], BF16, tag="msgB")
                    nc.vector.tensor_tensor(
                        out=msgB[:, :, 0:64].rearrange("p t (h c e) -> p t h c e",
                                                       h=H, e=2),
                        in0=gB[:, :, 0:64].rearrange("p t (h c e) -> p t h c e",
                                                     h=H, e=2),
                        in1=exHi2[:].unsqueeze(3).to_broadcast([128, uB, 4, 8, 2]),
                        op=ALU.mult)
                    nc.vector.tensor_copy(
                        out=msgB[:, :, 64:68],
                        in_=exHi[:].rearrange("p j u v -> p (j u) v"))

                    # --- aggregate per slot: 68-wide PSUM, 4 row blocks
                    U = pp.tile([128, 68], F32, tag="U")
                    for j in range(4):
                        for tt in range(nA):
                            nc.tensor.matmul(U[32 * j:32 * (j + 1), :],
                                             Sv[:, tt, j, :], msgA[:, j * nA + tt, :],
                                             start=(tt == 0), stop=False,
                                             tile_position=(0, 32 * j))
                        for tt in range(nB0, tps):
                            nc.tensor.matmul(U[32 * j:32 * (j + 1), :],
                                             Sv[:, tt, j, :],
                                             msgB[:, j * nBn + tt - nB0, :],
                                             start=False, stop=(tt == tps - 1),
                                             tile_position=(0, 32 * j))

                    # --- epilogue: out = U/(den+eps) + bias, relu
                    Usb = ep.tile([128, 64], F32, tag="Usb")
                    nc.vector.tensor_copy(out=Usb[:], in_=U[:, 0:64])
                    rec = ep.tile([128, 4], F32, tag="rec")
                    nc.vector.tensor_scalar_add(rec[:], U[:, 64:68], EPS)
                    nc.vector.reciprocal(rec[:], rec[:])
                    outq = ep.tile([128, 64], F32, tag="outq")
                    nc.vector.tensor_tensor(
                        out=outq[:].rearrange("p (h c) -> p h c", h=H),
                        in0=Usb[:].rearrange("p (h c) -> p h c", h=H),
                        in1=rec[:].unsqueeze(2).to_broadcast([128, H, HID]),
                        op=ALU.mult)
                    bias = b1bc if layer == 1 else b2bc
                    nc.vector.tensor_tensor(out=outq[:], in0=outq[:], in1=bias[:],
                                            op=ALU.add)
                    nc.vector.tensor_scalar_max(outq[:], outq[:], 0.0)

                    # transpose out_quad (both layers need it)
                    tp = p1.tile([128, 128], F32, tag="tp")
                    nc.tensor.transpose(tp[0:64, :], outq[:], Isb[:])
                    tpsb = ep.tile([64, 128], BF16, tag="tpsb")
                    nc.scalar.activation(tpsb[:], tp[0:64, :], ACT.Copy)

                    if layer == 1:
                        # layer-2 table rows + a_d2 for this quad's 128 nodes
                        t2ps = p1.tile([128, 72], F32, tag="t2ps")
                        nc.tensor.matmul(t2ps[:], tpsb[:], W2sb[:], start=True, stop=True)
                        t2sb = ep.tile([128, 68], BF16, tag="t2sb")
                        nc.scalar.activation(t2sb[:], t2ps[:, 0:68], ACT.Copy)
                        nc.vector.tensor_copy(out=ad2sb[:, q, :], in_=t2ps[:, 68:72])
                        nc.sync.dma_start(
                            d_T2loc.ap()[128 * q:128 * (q + 1), :], t2sb[:])
                    else:
                        # head: logits into zall; exp-sum into smsb
                        lg = p1.tile([128, OUT], F32, tag="lg")
                        nc.tensor.matmul(lg[:], tpsb[:], LWsb[:], start=True, stop=True)
                        nc.vector.tensor_tensor(out=zall[:, q, :], in0=lg[:],
                                                in1=lbbc[:], op=ALU.add)
                        mx = ep.tile([128, 1], F32, tag="mx")
                        nc.vector.reduce_max(mx[:], zall[:, q, :],
                                             axis=mybir.AxisListType.X)
                        nc.vector.tensor_scalar(out=zall[:, q, :], in0=zall[:, q, :],
                                                scalar1=mx[:], scalar2=None,
                                                op0=ALU.subtract)
                        ez = ep.tile([128, OUT], BF16, tag="ez")
                        nc.scalar.activation(ez[:], zall[:, q, :], ACT.Exp,
                                             accum_out=smsb[:, q:q + 1])

        edge_layer(1, d_T1, ad1sb)

        # one AllGather of the layer-2 table (device-major concat)
        nc.gpsimd.collective_compute(
            "AllGather", mybir.AluOpType.bypass,
            replica_groups=[list(range(C))],
            ins=[d_T2loc.ap()],
            outs=[d_T2t.ap()],
        )
        nc.sync.dma_start(d_T2.ap()[:, 0:68], d_T2t.ap()[:, :])

        edge_layer(2, d_T2, ad2sb)

        # ---------------- tail: log_softmax finish + single output DMA -------
        with ExitStack() as ph:
            tpool = ph.enter_context(tc.tile_pool(name="tail", bufs=1))
            lnall = tpool.tile([128, qpd], F32)
            nc.scalar.activation(lnall[:], smsb[:], ACT.Ln)
            nc.vector.tensor_tensor(
                out=zall[:],
                in0=zall[:],
                in1=lnall[:].unsqueeze(2).to_broadcast([128, qpd, OUT]),
                op=ALU.subtract)
            nc.sync.dma_start(t_out[:, :],
                              zall[:].rearrange("p q d -> p (q d)"))

    return nc


# ----------------------------------------------------------------------------
# public entry
# ----------------------------------------------------------------------------

def _prepare(inputs):
    x = np.asarray(inputs["x"], np.float32)
    ei = np.asarray(inputs["edge_index"], np.int64)
    ea = np.asarray(inputs["edge_attr"], np.float32)
    n = x.shape[0]
    loop = np.arange(n, dtype=np.int64)
    src = np.concatenate([ei[0], loop])
    dst = np.concatenate([ei[1], loop])
    mean_attr = ea.mean(axis=0)

    plan = _build_plan(src, dst, n)
    per_core, xT, tq = _host_arrays(plan, x, src, dst, ea, mean_attr, n)

    W1aug, W2aug, VeBD, LW, lb2p, b1, b2 = _fold_weights(
        np.asarray(inputs["W1"], np.float32), np.asarray(inputs["att_src1"], np.float32),
        np.asarray(inputs["att_dst1"], np.float32), np.asarray(inputs["We1"], np.float32),
        np.asarray(inputs["att_e1"], np.float32), np.asarray(inputs["b1"], np.float32),
        np.asarray(inputs["W2"], np.float32), np.asarray(inputs["att_src2"], np.float32),
        np.asarray(inputs["att_dst2"], np.float32), np.asarray(inputs["We2"], np.float32),
        np.asarray(inputs["att_e2"], np.float32), np.asarray(inputs["b2"], np.float32),
        np.asarray(inputs["lw1"], np.float32), np.asarray(inputs["lb1"], np.float32),
        np.asarray(inputs["lw2"], np.float32), np.asarray(inputs["lb2"], np.float32))

    bf16 = _bf16()
    cst = np.zeros((1, 256), np.float32)
    cst[0, 0:64] = b1
    cst[0, 64:128] = b2
    cst[0, 128:128 + OUT] = lb2p
    cst[0, 168:168 + NSLOT] = np.arange(NSLOT, dtype=np.float32)
    ident = np.eye(128, dtype=np.float32)
    bdmask = np.zeros((128, 16), np.float32)
    for j in range(4):
        bdmask[32 * j:32 * (j + 1), 4 * j:4 * (j + 1)] = 1.0
    bdmask = bdmask.astype(bf16)

    in_maps = []
    for d in range(C):
        pc = per_core[d]
        in_maps.append({
            "xT": xT, "xTloc": pc["xTloc"], "eaTg": pc["eaTg"], "comb": pc["comb"],
            "W1aug": W1aug, "W2aug": W2aug, "VeBD": VeBD, "LW": LW, "cst": cst,
            "ident": ident, "bdmask": bdmask,
        })
    return plan, in_maps, tq


def _assemble(plan, outs, n):
    bin_of, pos_of, spd, qpd = plan["bin_of"], plan["pos_of"], plan["spd"], plan["qpd"]
    dev = bin_of // spd
    s = bin_of % spd
    q = s // 4
    u = (s % 4) * NSLOT + pos_of
    stacked = np.stack([np.asarray(o, np.float32).reshape(128, qpd, OUT)
                        for o in outs])
    return stacked[dev[:n], u[:n], q[:n]]


def _run(inputs, trace=False, **spmd_kwargs):
    from concourse.bass_utils import run_bass_kernel_spmd

    plan, in_maps, tq = _prepare(inputs)
    nc = _build_nc(plan["ng"], plan["npd"], plan["spd"], plan["tps"], tq,
                   plan["nA"], plan["nB0"], plan["X"])
    nc.compile()
    res = run_bass_kernel_spmd(nc, in_maps, core_ids=list(range(C)), trace=trace,
                               **spmd_kwargs)
    outs = [r["out"] for r in res.results]
    return _assemble(plan, outs, inputs["x"].shape[0]), res


def kernel(**inputs):
    out, _ = _run(inputs)
    return out
